# revision 22
# baseline (speedup 1.0000x reference)
"""Trainium2 Bass kernel for nn_CombinedGraphLayer (LSH-binned GHConv message passing).

Contract: kernel(**inputs) takes FULL inputs (x [16,12800,256], msk [16,12800],
training scalar + weights), returns FULL output [16,12800,256].

Strategy: pure data-parallel over batch (2 batches per NeuronCore x 8 cores).
The wall clock is dominated by the ~55-75 MB/s host<->device tunnel, so the
pipeline is organized around minimizing wire bytes:

  put   x as bf16 (105MB instead of 210MB fp32)
  modK  (device): LSH argmax + top-2 gap per row from the bf16 input
  host  rows whose gap < TAU could have a different argmax than the fp32
        reference chain; recompute those exactly in fp64-free numpy fp32
        (~10% of rows, ~0.2s), then argsort -> exact per-row sort ranks
  modM  (device): layernorm -> ffn_dist -> pack, indirect-scatter rows into
        sorted bin order using the host ranks, then per 128-point bin:
        pairwise gaussian adjacency + 2 GHConv layers. Only the first NBU
        sorted bins are computed/emitted (all unmasked rows sort there);
        output rows leave in sorted order as bf16 + original-index column
  host  scatter rows back to input order (bf16 output quantization adds
        ~2e-3 max-rel error vs the 2e-2 tolerance)

Weights are folded (layernorm gamma/beta into the ffn/GHConv weights) and
embedded in the NEFF as constants - zero per-call wire cost.
"""

import hashlib
import numpy as np

import concourse.bass as bass
import concourse.tile as tile
from concourse import mybir
from concourse.masks import make_identity

dt = mybir.dt
OP = mybir.AluOpType
AF = mybir.ActivationFunctionType
IOA = bass.IndirectOffsetOnAxis

ABLATION = ""  # bench knob: "noB" (timing experiments only)

F = 256       # feature dim
D = 128       # distance dim
BIN = 128

# packed row layout (fp32): [ zm(0:256) | xd(256:384) | m(384) | idx(385) | pad ]
RW = 388
COL_M = 384
COL_IDX = 385

NBU = 54   # sorted 128-row bins computed per batch; all unmasked rows land in
           # the first ~nch/2+1 bins (msk ~ Bernoulli(0.5)); runtime-verified.
TAU = 1e-2  # risky-gap threshold; max |cmul(bf16 x) - cmul(fp32 x)| measured
            # at 2.2e-3, so 1e-2 leaves >4x margin. ~10% of rows get an exact
            # host recompute of their LSH argmax.


def split_excess_waits(nc):
    """This walrus build rejects instructions carrying more than a couple of
    sem waits (1 for CTRL-class like Drain, ~2 for compute). Move excess
    waits onto extra Drains inserted just before, on the same engine."""
    for f in nc.m.functions:
        for b in f.blocks:
            new_insts = []
            for inst in b.instructions:
                si = getattr(inst, "sync_info", None)
                ow = list(si.on_wait) if si is not None and si.on_wait else []
                limit = 1
                if len(ow) > limit and inst.engine is not None:
                    keep = ow[-limit:]
                    for w in ow[:-limit]:
                        d = mybir.InstNoOp(
                            name=nc.get_next_instruction_name(), ins=[], outs=[]
                        )
                        d.engine = inst.engine
                        d.sync_info = mybir.SyncInfo(on_wait=[w], on_update=[])
                        new_insts.append(d)
                    si.on_wait = keep
                new_insts.append(inst)
            b.instructions = new_insts


def _ffn_head(nc, pa, pap, wsb, ident, eps_t, ones_row_f, xb_chunk, m_chunk=None):
    """Shared LN -> ffn_dist chain for one 128-row chunk of bf16 input.
    Returns (z_t fp32 [128,F], xdT_sb fp32 [128,128] feature-major)."""
    f32 = dt.float32
    xb_t = pa.tile([128, F], dt.bfloat16)
    nc.sync.dma_start(out=xb_t[:], in_=xb_chunk)
    x_t = pa.tile([128, F], f32)
    nc.vector.tensor_copy(x_t[:], xb_t[:])

    st = pa.tile([128, 6], f32)
    nc.vector.bn_stats(out=st[:], in_=x_t[:])
    mv = pa.tile([128, 2], f32)
    nc.vector.bn_aggr(out=mv[:], in_=st[:])
    nc.scalar.activation(out=mv[:, 1:2], in_=mv[:, 1:2],
                         func=AF.Sqrt, bias=eps_t[:])
    nc.vector.reciprocal(out=mv[:, 1:2], in_=mv[:, 1:2])
    z_t = pa.tile([128, F], f32)
    nc.vector.tensor_scalar(
        out=z_t[:], in0=x_t[:], scalar1=mv[:, 0:1],
        scalar2=mv[:, 1:2], op0=OP.subtract, op1=OP.mult)

    # zT (feature-major) for the ffn matmuls
    zT_ps = pap.tile([128, 2, 128], f32, space="PSUM")
    for k in range(2):
        nc.tensor.transpose(zT_ps[:, k, :],
                            z_t[:, k * 128:(k + 1) * 128], ident[:])
    zT_sb = pa.tile([128, 2, 128], f32)
    nc.scalar.activation(out=zT_sb[:], in_=zT_ps[:], func=AF.Copy)

    # hT = W1g^T zT + b1gb  (feature-major [D, pts])
    h_ps = pap.tile([128, 128], f32, space="PSUM")
    nc.tensor.matmul(h_ps[:], lhsT=wsb["W1g"][:, 0, :],
                     rhs=zT_sb[:, 0, :], start=True, stop=False)
    nc.tensor.matmul(h_ps[:], lhsT=wsb["W1g"][:, 1, :],
                     rhs=zT_sb[:, 1, :], start=False, stop=False)
    nc.tensor.matmul(h_ps[:], lhsT=wsb["b1gb"][:],
                     rhs=ones_row_f[:], start=False, stop=True)
    # elu
    e_t = pa.tile([128, 128], f32)
    nc.vector.tensor_scalar_min(e_t[:], h_ps[:], 0.0)
    nc.scalar.activation(out=e_t[:], in_=e_t[:], func=AF.Exp)
    r_t = pa.tile([128, 128], f32)
    nc.scalar.activation(out=r_t[:], in_=h_ps[:], func=AF.Relu)
    hTe = pa.tile([128, 128], f32)
    nc.vector.scalar_tensor_tensor(
        out=hTe[:], in0=e_t[:], scalar=-1.0, in1=r_t[:],
        op0=OP.add, op1=OP.add)

    # xdT = W2^T hTe + b2
    xdT_ps = pap.tile([128, 128], f32, space="PSUM")
    nc.tensor.matmul(xdT_ps[:], lhsT=wsb["W2"][:], rhs=hTe[:],
                     start=True, stop=False)
    nc.tensor.matmul(xdT_ps[:], lhsT=wsb["b2"][:],
                     rhs=ones_row_f[:], start=False, stop=True)
    xdT_sb = pa.tile([128, 128], f32)
    nc.scalar.activation(out=xdT_sb[:], in_=xdT_ps[:], func=AF.Copy)
    return z_t, xdT_sb


def build_keys(nb, nch, w, tau):
    """modK: per-row LSH (argmax + 128*risky) packed as one uint8 output;
    risky = top-2 gap below tau."""
    NP = nch * BIN
    CB = nch // 2
    f32 = dt.float32
    nc = bass.Bass("TRN2", target_bir_lowering=False, debug=False)

    x_in = nc.dram_tensor("x", [nb * NP, F], dt.bfloat16,
                          kind="ExternalInput").ap()
    code_d = nc.dram_tensor("code", [nb * NP, 1], dt.uint8,
                            kind="ExternalOutput").ap()
    wdram = {n: nc.inline_tensor(w[n], name=n).ap()
             for n in ("W1g", "b1gb", "W2", "b2", "CB")}

    with tile.TileContext(nc) as tc:
        with tc.tile_pool(name="init", bufs=1) as ip:
            ident = ip.tile([128, 128], f32)
            make_identity(nc, ident[:])
            eps_t = ip.tile([128, 1], f32)
            nc.vector.memset(eps_t[:], 1e-6)
            ones_row_f = ip.tile([1, 128], f32)
            nc.vector.memset(ones_row_f[:], 1.0)
            tau_t = ip.tile([128, 1], f32)
            nc.vector.memset(tau_t[:], tau)
            wsb = {}
            for n in ("W1g", "b1gb", "W2", "b2", "CB"):
                s = list(w[n].shape)
                shp = [128, s[0] // 128, s[1]] if s[0] > 128 else s
                src = (wdram[n].rearrange("(c p) m -> p c m", p=128)
                       if s[0] > 128 else wdram[n][:])
                t = ip.tile(shp, f32, tag=f"w_{n}")
                nc.gpsimd.dma_start(out=t[:], in_=src)
                wsb[n] = t

            with tc.tile_pool(name="pk", bufs=3) as pa, \
                 tc.tile_pool(name="pkps", bufs=1, space="PSUM") as pap:
                for c in range(nb * nch):
                    row0 = c * 128
                    _, xdT_sb = _ffn_head(nc, pa, pap, wsb, ident, eps_t,
                                          ones_row_f,
                                          x_in[row0:row0 + 128, :])
                    # mul = xd @ codebook  (point-major [pts, CB])
                    mul_ps = pap.tile([128, CB], f32, space="PSUM")
                    nc.tensor.matmul(mul_ps[:], lhsT=xdT_sb[:], rhs=wsb["CB"][:],
                                     start=True, stop=True)
                    cmul = pa.tile([128, 2 * CB], f32)
                    nc.scalar.activation(out=cmul[:, 0:CB], in_=mul_ps[:],
                                         func=AF.Copy)
                    nc.scalar.activation(out=cmul[:, CB:2 * CB], in_=mul_ps[:],
                                         func=AF.Copy, scale=-1.0)
                    mx8 = pa.tile([128, 8], f32)
                    nc.vector.max(out=mx8[:], in_=cmul[:])
                    ix8 = pa.tile([128, 8], dt.uint32)
                    nc.vector.max_index(out=ix8[:], in_max=mx8[:],
                                        in_values=cmul[:])
                    idxf = pa.tile([128, 1], f32)
                    nc.vector.tensor_copy(idxf[:], ix8[:, 0:1])
                    # top-2 gap: mask out max positions, re-reduce
                    mxv = pa.tile([128, 1], f32)
                    nc.vector.tensor_reduce(out=mxv[:], in_=cmul[:],
                                            axis=mybir.AxisListType.X,
                                            op=OP.max)
                    eq = pa.tile([128, 2 * CB], f32)
                    nc.vector.tensor_scalar(
                        out=eq[:], in0=cmul[:], scalar1=mxv[:],
                        scalar2=None, op0=OP.is_equal)
                    c2 = pa.tile([128, 2 * CB], f32)
                    nc.vector.scalar_tensor_tensor(
                        out=c2[:], in0=eq[:], scalar=-1e30, in1=cmul[:],
                        op0=OP.mult, op1=OP.add)
                    mx2 = pa.tile([128, 1], f32)
                    nc.vector.tensor_reduce(out=mx2[:], in_=c2[:],
                                            axis=mybir.AxisListType.X,
                                            op=OP.max)
                    gap_t = pa.tile([128, 1], f32)
                    nc.vector.tensor_sub(gap_t[:], mxv[:], mx2[:])
                    # code = argmax + 128*(gap < tau), exact small ints;
                    # built from is_gt (known-good): am + 128 - 128*(gap>tau)
                    gt = pa.tile([128, 1], f32)
                    nc.vector.tensor_scalar(
                        out=gt[:], in0=gap_t[:], scalar1=tau_t[:],
                        scalar2=None, op0=OP.is_gt)
                    code_f = pa.tile([128, 1], f32)
                    nc.vector.scalar_tensor_tensor(
                        out=code_f[:], in0=gt[:], scalar=-128.0, in1=idxf[:],
                        op0=OP.mult, op1=OP.add)
                    nc.vector.tensor_scalar_add(code_f[:], code_f[:], 128.0)
                    code8 = pa.tile([128, 1], dt.uint8)
                    nc.vector.tensor_copy(code8[:], code_f[:])
                    nc.sync.dma_start(out=code_d[row0:row0 + 128, :],
                                      in_=code8[:])

    split_excess_waits(nc)
    return nc


def build(nb, nch, w, ghconv_dtype=dt.float32, nbu=None):
    """modM: full pipeline given host-computed sort ranks; bf16 x input."""
    NP = nch * BIN
    NBINS = nch
    if nbu is None:
        nbu = NBINS
    f32 = dt.float32
    bf16 = dt.bfloat16
    use_r = ghconv_dtype == dt.float32r
    gdt = ghconv_dtype

    nc = bass.Bass("TRN2", target_bir_lowering=False, debug=False)

    x_in = nc.dram_tensor("x", [nb * NP, F], bf16, kind="ExternalInput").ap()
    m_in = nc.dram_tensor("m", [nb * NP, 1], f32, kind="ExternalInput").ap()
    rank_in = nc.dram_tensor("rank", [nb * NP, 1], dt.uint32,
                             kind="ExternalInput").ap()
    wnames = ["W1g", "b1gb", "W2", "b2",
              "th0", "Wh0", "Wt0", "bth0", "bhh0", "bgt0",
              "th1", "Wh1", "Wt1", "bt1"]
    wdram = {n: nc.inline_tensor(w[n], name=n).ap() for n in wnames}
    outs = [nc.dram_tensor(f"out{b}", [nbu * BIN, F], dt.int8,
                           kind="ExternalOutput").ap()
            for b in range(nb)]
    osc = [nc.dram_tensor(f"osc{b}", [nbu * BIN, 1], f32,
                          kind="ExternalOutput").ap()
           for b in range(nb)]
    oidx = [nc.dram_tensor(f"oidx{b}", [nbu * BIN, 1], f32,
                           kind="ExternalOutput").ap()
            for b in range(nb)]
    psort = [nc.dram_tensor(f"psort{b}", [NP, RW], f32, kind="Internal").ap()
             for b in range(nb)]

    with tile.TileContext(nc) as tc:
        with tc.tile_pool(name="init", bufs=1) as ip:
            ident = ip.tile([128, 128], f32)
            make_identity(nc, ident[:])
            eps_t = ip.tile([128, 1], f32)
            nc.vector.memset(eps_t[:], 1e-6)
            iota_p_i = ip.tile([128, 1], dt.int32)
            nc.gpsimd.iota(iota_p_i[:], [[0, 1]], base=0, channel_multiplier=1)
            iota_p_f = ip.tile([128, 1], f32)
            nc.vector.tensor_copy(iota_p_f[:], iota_p_i[:])
            ones_row_f = ip.tile([1, 128], f32)
            nc.vector.memset(ones_row_f[:], 1.0)
            ones_row_g = ip.tile([1, 128], gdt)
            if gdt == f32:
                nc.vector.memset(ones_row_g[:], 1.0)
            else:
                nc.vector.tensor_copy(ones_row_g[:], ones_row_f[:])

            # weights to SBUF
            wsb = {}
            for n in wnames:
                s = list(w[n].shape)
                wdt = f32
                if n in ("th0", "Wh0", "Wt0", "th1", "Wh1", "Wt1",
                         "bth0", "bhh0", "bgt0", "bt1"):
                    wdt = gdt
                shp = [128, s[0] // 128, s[1]] if s[0] > 128 else s
                src = (wdram[n].rearrange("(c p) m -> p c m", p=128)
                       if s[0] > 128 else wdram[n][:])
                if wdt == f32:
                    t = ip.tile(shp, f32, tag=f"w_{n}")
                    nc.gpsimd.dma_start(out=t[:], in_=src)
                else:
                    stg = ip.tile(shp, f32, tag="w_stage")
                    nc.gpsimd.dma_start(out=stg[:], in_=src)
                    t = ip.tile(shp, wdt, tag=f"w_{n}")
                    nc.vector.tensor_copy(t[:], stg[:])
                wsb[n] = t

            for b in range(nb):
                _one_batch(tc, nc, b, nb, nch, NP, NBINS, nbu,
                           x_in, m_in, rank_in, wsb, outs[b], osc[b], oidx[b],
                           psort[b], ident, eps_t, iota_p_f,
                           ones_row_f, ones_row_g, gdt, use_r)

    split_excess_waits(nc)
    return nc


def _one_batch(tc, nc, b, nb, nch, NP, NBINS, nbu,
               x_in, m_in, rank_in, wsb, out_d, osc_d, oidx_d, psort_d,
               ident, eps_t, iota_p_f, ones_row_f, ones_row_g, gdt, use_r):
    f32 = dt.float32
    bf16 = dt.bfloat16
    if use_r:
        def R(ap):
            return ap.bitcast(dt.float32r)
    else:
        def R(ap):
            return ap

    with tc.tile_pool(name=f"res{b}", bufs=1) as rp:
        packed = rp.tile([128, nch, RW], f32)     # resident z*m / xd / m / idx
        rank_u = rp.tile([128, nch], dt.uint32)

        # ---------------- phase A: LN -> ffn -> pack ----------------
        with tc.tile_pool(name=f"pa{b}", bufs=3) as pa, \
             tc.tile_pool(name=f"paps{b}", bufs=1, space="PSUM") as pap:
            for c in range(nch):
                row0 = b * NP + c * 128
                nc.sync.dma_start(out=packed[:, c, COL_M:COL_M + 1],
                                  in_=m_in[row0:row0 + 128, :])
                nc.sync.dma_start(out=rank_u[:, c:c + 1],
                                  in_=rank_in[row0:row0 + 128, :])
                m_ap = packed[:, c, COL_M:COL_M + 1]
                z_t, xdT_sb = _ffn_head(nc, pa, pap, wsb, ident, eps_t,
                                        ones_row_f, x_in[row0:row0 + 128, :])
                # zm into packed (gpsimd: SBUF only)
                nc.gpsimd.tensor_scalar_mul(packed[:, c, 0:F], z_t[:], m_ap)
                # xd point-major into packed
                xd_ps = pap.tile([128, 128], f32, space="PSUM")
                nc.tensor.transpose(xd_ps[:], xdT_sb[:], ident[:])
                nc.vector.tensor_copy(packed[:, c, F:F + 128], xd_ps[:])
                # idx column
                nc.vector.tensor_scalar_add(
                    packed[:, c, COL_IDX:COL_IDX + 1], iota_p_f[:],
                    float(c * 128))

        # ---------------- phase A2: scatter rows to sorted order ----------
        for c in range(nch):
            nc.gpsimd.indirect_dma_start(
                out=psort_d[:],
                out_offset=IOA(ap=rank_u[:, c:c + 1], axis=0),
                in_=packed[:, c, :], in_offset=None)

    # ---------------- phase B: adjacency + GHConv per bin ----------------
    if "noB" in ABLATION:
        return
    with tc.tile_pool(name=f"pb{b}", bufs=4) as pb, \
         tc.tile_pool(name=f"pbps{b}", bufs=1, space="PSUM") as pbp:
        for s in range(nbu):
            pk = pb.tile([128, RW], f32)
            nc.sync.dma_start(out=pk[:], in_=psort_d[s * 128:(s + 1) * 128, :])
            m_ap = pk[:, COL_M:COL_M + 1]
            # V cols: [na, one, one, na, m]; transposed pair/row tiles all
            # land at partition base 0 (matmul requires equal bases).
            V = pb.tile([128, 5], f32)
            sq = pb.tile([128, 128], f32)
            nc.scalar.activation(out=sq[:], in_=pk[:, F:F + 128],
                                 func=AF.Square, accum_out=V[:, 0:1])
            nc.gpsimd.memset(V[:, 1:3], 1.0)
            nc.gpsimd.tensor_copy(V[:, 3:4], V[:, 0:1])
            nc.gpsimd.tensor_copy(V[:, 4:5], m_ap)
            vt_ps = pbp.tile([2, 384], f32, space="PSUM")
            nc.tensor.transpose(vt_ps[0:2, 0:128], V[:, 0:2], ident[:])
            VTa = pb.tile([2, 128], f32)
            nc.scalar.activation(out=VTa[:], in_=vt_ps[0:2, 0:128],
                                 func=AF.Copy)
            nc.tensor.transpose(vt_ps[0:2, 128:256], V[:, 2:4], ident[:])
            VTb = pb.tile([2, 128], f32)
            nc.scalar.activation(out=VTb[:], in_=vt_ps[0:2, 128:256],
                                 func=AF.Copy)
            nc.tensor.transpose(vt_ps[0:1, 256:384], V[:, 4:5], ident[:])
            mT_sb = pb.tile([1, 128], f32)
            nc.scalar.activation(out=mT_sb[:], in_=vt_ps[0:1, 256:384],
                                 func=AF.Copy)
            # d2 = na_i - 2 xd xd^T + na_j ; M2 = m_i m_j
            adj_ps = pbp.tile([128, 384], f32, space="PSUM")
            xdT_ps = adj_ps[:, 0:128]
            d2_ps = adj_ps[:, 128:256]
            M2_ps = adj_ps[:, 256:384]
            nc.tensor.transpose(xdT_ps, pk[:, F:F + 128], ident[:])
            xdT = pb.tile([128, 128], f32)
            nc.scalar.activation(out=xdT[:], in_=xdT_ps, func=AF.Copy)
            xdTm2 = pb.tile([128, 128], f32)
            nc.scalar.activation(out=xdTm2[:], in_=xdT_ps, func=AF.Copy,
                                 scale=-2.0)
            nc.tensor.matmul(d2_ps, lhsT=xdTm2[:], rhs=xdT[:],
                             start=True, stop=False)
            nc.tensor.matmul(d2_ps, lhsT=VTa[:], rhs=VTb[:],
                             start=False, stop=True)
            nc.tensor.matmul(M2_ps, lhsT=mT_sb[:], rhs=mT_sb[:],
                             start=True, stop=True)
            dsc = pb.tile([128, 128], f32)
            nc.vector.tensor_scalar_max(dsc[:], d2_ps[:], 1e-6)
            nc.scalar.activation(out=dsc[:], in_=dsc[:], func=AF.Sqrt)
            nc.scalar.activation(out=dsc[:], in_=dsc[:], func=AF.Exp,
                                 scale=-0.1)
            dm = pb.tile([128, 128], gdt)
            ind = pb.tile([128, 1], f32)
            nc.vector.scalar_tensor_tensor(
                out=dm[:], in0=dsc[:], scalar=1.0, in1=M2_ps[:],
                op0=OP.mult, op1=OP.mult, accum_out=ind[:])
            nrm = pb.tile([128, 1], f32)
            nc.scalar.activation(out=nrm[:], in_=ind[:], func=AF.Sqrt,
                                 bias=eps_t[:])
            nc.vector.reciprocal(nrm[:], nrm[:])
            nc.vector.tensor_mul(nrm[:], nrm[:], m_ap)

            xb_ap = pk[:, 0:F]
            for li in range(2):
                sfx = "0" if li == 0 else "1"
                mm1 = pbp.tile([128, 512], f32, space="PSUM")
                mm2 = pbp.tile([128, 512], f32, space="PSUM")
                gat_ps = pbp.tile([128, F], f32, space="PSUM")
                xmT_ps = mm1[:, 0:256]
                hom2_ps = mm1[:, 256:512]
                hom_ps = mm2[:, 0:256]
                het_ps = mm2[:, 256:512]
                for k in range(2):
                    nc.tensor.transpose(
                        xmT_ps.rearrange("p (c q) -> p c q", q=128)[:, k, :],
                        xb_ap[:, k * 128:(k + 1) * 128], ident[:])
                xmT = pb.tile([128, 2, 128], gdt)
                nc.scalar.activation(out=xmT[:], in_=xmT_ps, func=AF.Copy)
                mT = mT_sb[:]
                if gdt != f32:
                    mTg = pb.tile([1, 128], gdt)
                    nc.vector.tensor_copy(mTg[:], mT_sb[:])
                    mT = mTg[:]
                # keep each PSUM accumulation group's matmuls consecutive
                for dst, wn, bias in (
                    (hom_ps, "th" + sfx, "bth0" if li == 0 else None),
                    (het_ps, "Wh" + sfx, "bhh0" if li == 0 else None),
                    (gat_ps[:], "Wt" + sfx,
                     "bgt0" if li == 0 else "bt1"),
                ):
                    for k in range(2):
                        nc.tensor.matmul(
                            dst, lhsT=R(xmT[:, k, :]), rhs=R(wsb[wn][:, k, :]),
                            start=(k == 0), stop=(k == 1 and bias is None))
                    if bias is not None:
                        blhs = mT if li == 0 else ones_row_g[:]
                        nc.tensor.matmul(dst, lhsT=R(blhs), rhs=R(wsb[bias][:]),
                                         start=False, stop=True)
                fh1 = pb.tile([128, F], gdt)
                nc.vector.tensor_scalar_mul(fh1[:], hom_ps[:], nrm[:])
                nc.tensor.matmul(hom2_ps[:], lhsT=R(dm[:]), rhs=R(fh1[:]),
                                 start=True, stop=True)
                gate = pb.tile([128, F], f32)
                nc.scalar.activation(out=gate[:], in_=gat_ps[:], func=AF.Sigmoid)
                fh2 = pb.tile([128, F], f32)
                nc.vector.tensor_scalar_mul(fh2[:], hom2_ps[:], nrm[:])
                nc.vector.tensor_sub(fh2[:], fh2[:], het_ps[:])
                nc.vector.tensor_mul(gate[:], gate[:], fh2[:])
                nc.vector.tensor_add(fh2[:], gate[:], het_ps[:])  # pre-act
                emin = pb.tile([128, F], f32)
                nc.gpsimd.tensor_scalar_min(emin[:], fh2[:], 0.0)
                nc.scalar.activation(out=emin[:], in_=emin[:], func=AF.Exp)
                er = pb.tile([128, F], f32)
                nc.scalar.activation(out=er[:], in_=fh2[:], func=AF.Relu)
                nc.vector.scalar_tensor_tensor(
                    out=emin[:], in0=emin[:], scalar=-1.0, in1=er[:],
                    op0=OP.add, op1=OP.add)
                out_t = pb.tile([128, F], f32)
                nc.gpsimd.tensor_scalar_mul(out_t[:], emin[:], m_ap)
                xb_ap = out_t[:]
            # emit sorted-order rows as int8 with a per-row fp32 scale
            # (rowmax/126.5 so the +0.5*sign rounding bias can never
            # saturate past 127) + original-index column; the host
            # dequantizes and scatters rows back to input order
            rabs = pb.tile([128, 1], f32)
            nc.vector.tensor_reduce(out=rabs[:], in_=xb_ap,
                                    axis=mybir.AxisListType.X, op=OP.max,
                                    apply_absolute_value=True)
            scq = pb.tile([128, 1], f32)
            nc.scalar.activation(out=scq[:], in_=rabs[:], func=AF.Copy,
                                 scale=1.0 / 126.5)
            nc.sync.dma_start(out=osc_d[s * 128:(s + 1) * 128, :], in_=scq[:])
            rc = pb.tile([128, 1], f32)
            nc.vector.tensor_scalar_max(rc[:], rabs[:], 1e-30)
            inv = pb.tile([128, 1], f32)
            nc.vector.reciprocal(inv[:], rc[:])
            inv127 = pb.tile([128, 1], f32)
            nc.scalar.activation(out=inv127[:], in_=inv[:], func=AF.Copy,
                                 scale=126.5)
            qf = pb.tile([128, F], f32)
            nc.vector.tensor_scalar_mul(qf[:], xb_ap, inv127[:])
            # round-to-nearest: add +-0.5 via (q>0)-0.5, then int convert
            sg = pb.tile([128, F], f32)
            nc.vector.tensor_scalar(
                out=sg[:], in0=qf[:], scalar1=0.0, scalar2=None, op0=OP.is_gt)
            nc.vector.scalar_tensor_tensor(
                out=qf[:], in0=sg[:], scalar=-0.5, in1=qf[:],
                op0=OP.add, op1=OP.add)
            q8 = pb.tile([128, F], dt.int8)
            nc.vector.tensor_copy(q8[:], qf[:])
            nc.sync.dma_start(out=out_d[s * 128:(s + 1) * 128, :], in_=q8[:])
            nc.sync.dma_start(out=oidx_d[s * 128:(s + 1) * 128, :],
                              in_=pk[:, COL_IDX:COL_IDX + 1])


def _fold_weights(inputs):
    g = inputs["ln_gamma"].astype(np.float32)
    be = inputs["ln_beta"].astype(np.float32)
    W1 = inputs["W1"].astype(np.float32)
    b1 = inputs["b1"].astype(np.float32)
    w = {
        "W1g": g[:, None] * W1,
        "b1gb": (b1 + be @ W1)[None, :],
        "W2": inputs["W2"].astype(np.float32),
        "b2": inputs["b2"].astype(np.float32)[None, :],
        "th1": inputs["th1"].astype(np.float32),
        "Wh1": inputs["Wh1"].astype(np.float32),
        "Wt1": inputs["Wt1"].astype(np.float32),
        "bt1": inputs["bt1"].astype(np.float32)[None, :],
    }
    for nm in ("th0", "Wh0", "Wt0"):
        w[nm] = g[:, None] * inputs[nm].astype(np.float32)
    w["bth0"] = (be @ inputs["th0"].astype(np.float32))[None, :]
    w["bhh0"] = (be @ inputs["Wh0"].astype(np.float32))[None, :]
    w["bgt0"] = (inputs["bt0"].astype(np.float32) +
                 be @ inputs["Wt0"].astype(np.float32))[None, :]
    return {k: np.ascontiguousarray(v, dtype=np.float32) for k, v in w.items()}


_RUNNER_CACHE = {}


def _make_runner(nc, n_cores):
    """Jit a Bass module for SPMD execution; returns the callable + metadata."""
    import jax
    from jax.sharding import Mesh, PartitionSpec, NamedSharding
    from jax.experimental.shard_map import shard_map
    from concourse import bass2jax

    partition_name = (nc.partition_id_tensor.name
                      if nc.partition_id_tensor else None)
    in_names, out_names, out_avals, zero_shapes = [], [], [], []
    for alloc in nc.m.functions[0].allocations:
        if not isinstance(alloc, mybir.MemoryLocationSet):
            continue
        name = alloc.memorylocations[0].name
        if alloc.kind == "ExternalInput":
            if name != partition_name:
                in_names.append(name)
        elif alloc.kind == "ExternalOutput":
            out_names.append(name)
            shape = tuple(alloc.tensor_shape)
            dtype = mybir.dt.np(alloc.dtype)
            out_avals.append(jax.core.ShapedArray(shape, dtype))
            zero_shapes.append((shape, dtype))
    n_params = len(in_names)
    all_names = in_names + out_names
    if partition_name is not None:
        all_names = all_names + [partition_name]

    def _body(*args):
        operands = list(args)
        if partition_name is not None:
            operands.append(bass2jax.partition_id_tensor())
        outs = bass2jax._bass_exec_p.bind(
            *operands,
            out_avals=tuple(out_avals),
            in_names=tuple(all_names),
            out_names=tuple(out_names),
            lowering_input_output_aliases=(),
            sim_require_finite=True,
            sim_require_nnan=True,
            nc=nc,
        )
        return tuple(outs)

    devices = jax.devices()[:n_cores]
    mesh = Mesh(np.asarray(devices), ("core",))
    in_specs = (PartitionSpec("core"),) * (n_params + len(out_names))
    out_specs = (PartitionSpec("core"),) * len(out_names)
    sharded = jax.jit(
        shard_map(_body, mesh=mesh, in_specs=in_specs, out_specs=out_specs,
                  check_rep=False),
        keep_unused=True)
    # zero output buffers staged on device ONCE and reused read-only
    shard = NamedSharding(mesh, PartitionSpec("core"))
    dev_zeros = [
        jax.device_put(np.zeros((n_cores * s0[0], *s0[1:]), d), shard)
        for s0, d in zero_shapes]
    return (sharded, in_names, out_names, out_avals, dev_zeros)


def _get_runners(nb, nch, ghconv_dtype, n_cores, nbu, w):
    """Cached (modK, modM) runners; weights are compile-time constants, so
    the cache key includes their fingerprint."""
    wkey = hashlib.blake2b(
        b"".join(w[k].tobytes() for k in sorted(w)), digest_size=16).hexdigest()
    key = (nb, nch, ghconv_dtype, n_cores, nbu, wkey, ABLATION)
    if key not in _RUNNER_CACHE:
        from concourse import bass2jax
        bass2jax.install_neuronx_cc_hook()
        ncK = build_keys(nb, nch, w, TAU)
        ncM = build(nb, nch, w, ghconv_dtype, nbu=nbu)
        _RUNNER_CACHE[key] = (_make_runner(ncK, n_cores),
                              _make_runner(ncM, n_cores))
    return _RUNNER_CACHE[key]


def _host_fix_keys(x2d, ridx, w, nbins):
    """Exact fp32 LSH argmax for the given row indices (matches the
    reference chain: LN (gamma/beta folded) -> ffn_dist -> argmax)."""
    xr = x2d[ridx].astype(np.float32)
    mu = xr.mean(-1, keepdims=True)
    var = ((xr - mu) ** 2).mean(-1, keepdims=True)
    zn = (xr - mu) / np.sqrt(var + 1e-6)
    h = zn @ w["W1g"] + w["b1gb"]
    h = np.where(h > 0, h, np.expm1(np.minimum(h, 0)))
    xd = h @ w["W2"] + w["b2"]
    mul = xd @ w["CB"]
    cmul = np.concatenate([mul, -mul], -1)
    return np.argmax(cmul, -1)


def run(inputs, nb, nch, n_cores, ghconv_dtype=dt.float32, trace=False,
        nbu=NBU):
    """inputs: dict with x [Btot, NP, F] float32, msk [Btot, NP] bool + weights.
    Btot must equal n_cores * nb."""
    import concurrent.futures as cf
    import jax
    from jax.sharding import Mesh, PartitionSpec, NamedSharding
    import ml_dtypes

    NP = nch * BIN
    NBINS = nch
    x = np.ascontiguousarray(inputs["x"], dtype=np.float32)
    msk = np.asarray(inputs["msk"])
    Btot = x.shape[0]
    assert Btot == n_cores * nb
    w = _fold_weights(inputs)
    w["CB"] = np.ascontiguousarray(
        inputs["codebook"][:, :NBINS // 2], dtype=np.float32)

    (rK, rM) = _get_runners(nb, nch, ghconv_dtype, n_cores, nbu, w)
    shardedK, in_namesK, out_namesK, _, dev_zerosK = rK
    shardedM, in_namesM, out_namesM, _, dev_zerosM = rM

    # ---- put: x once as bf16, shared by both dispatches ----
    x2d = x.reshape(Btot * NP, F)
    xb = x2d.astype(ml_dtypes.bfloat16)
    mf = msk.astype(np.float32).reshape(Btot * NP, 1)
    mesh = Mesh(np.asarray(jax.devices()[:n_cores]), ("core",))
    shard = NamedSharding(mesh, PartitionSpec("core"))
    xb_dev = jax.device_put(xb, shard)

    # ---- modK: LSH argmax + risky bit from bf16 x ----
    full = {"x": xb_dev}
    outK = shardedK(*[full[n] for n in in_namesK], *dev_zerosK)
    resK = dict(zip(out_namesK, outK))
    code = np.asarray(resK["code"]).reshape(Btot * NP)
    am = (code & 127).astype(np.int32)

    # ---- host: exact argmax for risky rows, then sort ranks ----
    ridx = np.nonzero(code >= 128)[0]
    if len(ridx):
        am[ridx] = _host_fix_keys(x2d, ridx, w, NBINS)
    keys = am.reshape(Btot, NP) + np.where(~msk, NBINS - 1, 0)
    perm = np.argsort(keys, axis=-1, kind="stable")
    ranks = np.empty((Btot, NP), np.uint32)
    ar = np.arange(NP, dtype=np.uint32)
    for bi in range(Btot):
        ranks[bi, perm[bi]] = ar

    # ---- modM: main pipeline with exact ranks ----
    full = {"x": xb_dev, "m": mf, "rank": ranks.reshape(Btot * NP, 1)}
    outM = shardedM(*[full[n] for n in in_namesM], *dev_zerosM)
    resM = dict(zip(out_namesM, outM))
    with cf.ThreadPoolExecutor(len(outM)) as ex:
        host_outs = list(ex.map(np.asarray, outM))
    resM = dict(zip(out_namesM, host_outs))

    # ---- host: dequantize + scatter sorted rows back to input order ----
    out = np.zeros((Btot, NP, F), np.float32)
    for core in range(n_cores):
        for b in range(nb):
            gb = core * nb + b
            q = resM[f"out{b}"].reshape(n_cores, nbu * BIN, F)[core]
            sc = resM[f"osc{b}"].reshape(n_cores, nbu * BIN, 1)[core]
            ids = resM[f"oidx{b}"].reshape(n_cores, nbu * BIN)[core]
            ids = ids.astype(np.int64)
            out[gb, ids] = q.astype(np.float32) * sc
            # every unmasked row must have been emitted within the prefix
            covered = np.zeros(NP, bool)
            covered[ids] = True
            if not (covered | ~msk[gb]).all():
                raise RuntimeError(
                    f"batch {gb}: unmasked rows beyond {nbu} sorted bins; "
                    f"increase NBU")
    return out, None


def kernel(**inputs):
    out, _ = run(inputs, nb=2, nch=100, n_cores=8)
    return out


# revision 29
# speedup vs baseline: 1.3982x; 1.3982x over previous
"""Trainium2 Bass kernel for nn_CombinedGraphLayer (LSH-binned GHConv message passing).

Contract: kernel(**inputs) takes FULL inputs (x [16,12800,256], msk [16,12800],
training scalar + weights), returns FULL output [16,12800,256].

Strategy: pure data-parallel over batch (2 batches per NeuronCore x 8 cores).
The wall clock is dominated by the ~55-75 MB/s host<->device tunnel, so the
pipeline is organized around minimizing wire bytes:

  put   x as bf16 (105MB instead of 210MB fp32)
  modK  (device): LSH argmax + top-2 gap per row from the bf16 input
  host  rows whose gap < TAU could have a different argmax than the fp32
        reference chain; recompute those exactly in fp64-free numpy fp32
        (~10% of rows, ~0.2s), then argsort -> exact per-row sort ranks
  modM  (device): layernorm -> ffn_dist -> pack, indirect-scatter rows into
        sorted bin order using the host ranks, then per 128-point bin:
        pairwise gaussian adjacency + 2 GHConv layers. Only the first NBU
        sorted bins are computed/emitted (all unmasked rows sort there);
        output rows leave in sorted order as bf16 + original-index column
  host  scatter rows back to input order (bf16 output quantization adds
        ~2e-3 max-rel error vs the 2e-2 tolerance)

Weights are folded (layernorm gamma/beta into the ffn/GHConv weights) and
embedded in the NEFF as constants - zero per-call wire cost.
"""

import hashlib
import numpy as np

import concourse.bass as bass
import concourse.tile as tile
from concourse import mybir
from concourse.masks import make_identity

dt = mybir.dt
OP = mybir.AluOpType
AF = mybir.ActivationFunctionType
IOA = bass.IndirectOffsetOnAxis

ABLATION = ""  # bench knob: "noB" (timing experiments only)

F = 256       # feature dim
D = 128       # distance dim
BIN = 128

# packed row layout (fp32): [ zm(0:256) | xd(256:384) | m(384) | idx(385) | pad ]
RW = 388
COL_M = 384
COL_IDX = 385

NBU = 54   # sorted 128-row bins computed per batch; all unmasked rows land in
           # the first ~nch/2+1 bins (msk ~ Bernoulli(0.5)); runtime-verified.
TAU = 5e-3  # risky-gap threshold; max |cmul(bf16 x) - cmul(fp32 x)| measured
            # at 2.2e-3, so 5e-3 leaves >2x margin (zero non-risky flips seen
            # even at 4.3e-3). ~6% of rows get an exact host recompute of
            # their LSH argmax.


def split_excess_waits(nc):
    """This walrus build rejects instructions carrying more than a couple of
    sem waits (1 for CTRL-class like Drain, ~2 for compute). Move excess
    waits onto extra Drains inserted just before, on the same engine."""
    for f in nc.m.functions:
        for b in f.blocks:
            new_insts = []
            for inst in b.instructions:
                si = getattr(inst, "sync_info", None)
                ow = list(si.on_wait) if si is not None and si.on_wait else []
                limit = 1
                if len(ow) > limit and inst.engine is not None:
                    keep = ow[-limit:]
                    for w in ow[:-limit]:
                        d = mybir.InstNoOp(
                            name=nc.get_next_instruction_name(), ins=[], outs=[]
                        )
                        d.engine = inst.engine
                        d.sync_info = mybir.SyncInfo(on_wait=[w], on_update=[])
                        new_insts.append(d)
                    si.on_wait = keep
                new_insts.append(inst)
            b.instructions = new_insts


def _ffn_head(nc, pa, pap, wsb, ident, eps_t, ones_row_f, xb_chunk, m_chunk=None):
    """Shared LN -> ffn_dist chain for one 128-row chunk of bf16 input.
    Returns (z_t fp32 [128,F], xdT_sb fp32 [128,128] feature-major)."""
    f32 = dt.float32
    xb_t = pa.tile([128, F], dt.bfloat16)
    nc.sync.dma_start(out=xb_t[:], in_=xb_chunk)
    x_t = pa.tile([128, F], f32)
    nc.vector.tensor_copy(x_t[:], xb_t[:])

    st = pa.tile([128, 6], f32)
    nc.vector.bn_stats(out=st[:], in_=x_t[:])
    mv = pa.tile([128, 2], f32)
    nc.vector.bn_aggr(out=mv[:], in_=st[:])
    nc.scalar.activation(out=mv[:, 1:2], in_=mv[:, 1:2],
                         func=AF.Sqrt, bias=eps_t[:])
    nc.vector.reciprocal(out=mv[:, 1:2], in_=mv[:, 1:2])
    z_t = pa.tile([128, F], f32)
    nc.vector.tensor_scalar(
        out=z_t[:], in0=x_t[:], scalar1=mv[:, 0:1],
        scalar2=mv[:, 1:2], op0=OP.subtract, op1=OP.mult)

    # zT (feature-major) for the ffn matmuls
    zT_ps = pap.tile([128, 2, 128], f32, space="PSUM")
    for k in range(2):
        nc.tensor.transpose(zT_ps[:, k, :],
                            z_t[:, k * 128:(k + 1) * 128], ident[:])
    zT_sb = pa.tile([128, 2, 128], f32)
    nc.scalar.activation(out=zT_sb[:], in_=zT_ps[:], func=AF.Copy)

    # hT = W1g^T zT + b1gb  (feature-major [D, pts])
    h_ps = pap.tile([128, 128], f32, space="PSUM")
    nc.tensor.matmul(h_ps[:], lhsT=wsb["W1g"][:, 0, :],
                     rhs=zT_sb[:, 0, :], start=True, stop=False)
    nc.tensor.matmul(h_ps[:], lhsT=wsb["W1g"][:, 1, :],
                     rhs=zT_sb[:, 1, :], start=False, stop=False)
    nc.tensor.matmul(h_ps[:], lhsT=wsb["b1gb"][:],
                     rhs=ones_row_f[:], start=False, stop=True)
    # elu
    e_t = pa.tile([128, 128], f32)
    nc.vector.tensor_scalar_min(e_t[:], h_ps[:], 0.0)
    nc.scalar.activation(out=e_t[:], in_=e_t[:], func=AF.Exp)
    r_t = pa.tile([128, 128], f32)
    nc.scalar.activation(out=r_t[:], in_=h_ps[:], func=AF.Relu)
    hTe = pa.tile([128, 128], f32)
    nc.vector.scalar_tensor_tensor(
        out=hTe[:], in0=e_t[:], scalar=-1.0, in1=r_t[:],
        op0=OP.add, op1=OP.add)

    # xdT = W2^T hTe + b2
    xdT_ps = pap.tile([128, 128], f32, space="PSUM")
    nc.tensor.matmul(xdT_ps[:], lhsT=wsb["W2"][:], rhs=hTe[:],
                     start=True, stop=False)
    nc.tensor.matmul(xdT_ps[:], lhsT=wsb["b2"][:],
                     rhs=ones_row_f[:], start=False, stop=True)
    xdT_sb = pa.tile([128, 128], f32)
    nc.scalar.activation(out=xdT_sb[:], in_=xdT_ps[:], func=AF.Copy)
    return z_t, xdT_sb


def build_keys(nb, nch, w, tau):
    """modK: per-row LSH (argmax + 128*risky) packed as one uint8 output;
    risky = top-2 gap below tau."""
    NP = nch * BIN
    CB = nch // 2
    f32 = dt.float32
    nc = bass.Bass("TRN2", target_bir_lowering=False, debug=False)

    x_in = nc.dram_tensor("x", [nb * NP, F], dt.bfloat16,
                          kind="ExternalInput").ap()
    code_d = nc.dram_tensor("code", [nb * NP, 1], dt.uint8,
                            kind="ExternalOutput").ap()
    wdram = {n: nc.inline_tensor(w[n], name=n).ap()
             for n in ("W1g", "b1gb", "W2", "b2", "CB")}

    with tile.TileContext(nc) as tc:
        with tc.tile_pool(name="init", bufs=1) as ip:
            ident = ip.tile([128, 128], f32)
            make_identity(nc, ident[:])
            eps_t = ip.tile([128, 1], f32)
            nc.vector.memset(eps_t[:], 1e-6)
            ones_row_f = ip.tile([1, 128], f32)
            nc.vector.memset(ones_row_f[:], 1.0)
            tau_t = ip.tile([128, 1], f32)
            nc.vector.memset(tau_t[:], tau)
            wsb = {}
            for n in ("W1g", "b1gb", "W2", "b2", "CB"):
                s = list(w[n].shape)
                shp = [128, s[0] // 128, s[1]] if s[0] > 128 else s
                src = (wdram[n].rearrange("(c p) m -> p c m", p=128)
                       if s[0] > 128 else wdram[n][:])
                t = ip.tile(shp, f32, tag=f"w_{n}")
                nc.gpsimd.dma_start(out=t[:], in_=src)
                wsb[n] = t

            with tc.tile_pool(name="pk", bufs=3) as pa, \
                 tc.tile_pool(name="pkps", bufs=1, space="PSUM") as pap:
                for c in range(nb * nch):
                    row0 = c * 128
                    _, xdT_sb = _ffn_head(nc, pa, pap, wsb, ident, eps_t,
                                          ones_row_f,
                                          x_in[row0:row0 + 128, :])
                    # mul = xd @ codebook  (point-major [pts, CB])
                    mul_ps = pap.tile([128, CB], f32, space="PSUM")
                    nc.tensor.matmul(mul_ps[:], lhsT=xdT_sb[:], rhs=wsb["CB"][:],
                                     start=True, stop=True)
                    cmul = pa.tile([128, 2 * CB], f32)
                    nc.scalar.activation(out=cmul[:, 0:CB], in_=mul_ps[:],
                                         func=AF.Copy)
                    nc.scalar.activation(out=cmul[:, CB:2 * CB], in_=mul_ps[:],
                                         func=AF.Copy, scale=-1.0)
                    mx8 = pa.tile([128, 8], f32)
                    nc.vector.max(out=mx8[:], in_=cmul[:])
                    ix8 = pa.tile([128, 8], dt.uint32)
                    nc.vector.max_index(out=ix8[:], in_max=mx8[:],
                                        in_values=cmul[:])
                    idxf = pa.tile([128, 1], f32)
                    nc.vector.tensor_copy(idxf[:], ix8[:, 0:1])
                    # top-2 gap: mask out max positions, re-reduce
                    mxv = pa.tile([128, 1], f32)
                    nc.vector.tensor_reduce(out=mxv[:], in_=cmul[:],
                                            axis=mybir.AxisListType.X,
                                            op=OP.max)
                    eq = pa.tile([128, 2 * CB], f32)
                    nc.vector.tensor_scalar(
                        out=eq[:], in0=cmul[:], scalar1=mxv[:],
                        scalar2=None, op0=OP.is_equal)
                    c2 = pa.tile([128, 2 * CB], f32)
                    nc.vector.scalar_tensor_tensor(
                        out=c2[:], in0=eq[:], scalar=-1e30, in1=cmul[:],
                        op0=OP.mult, op1=OP.add)
                    mx2 = pa.tile([128, 1], f32)
                    nc.vector.tensor_reduce(out=mx2[:], in_=c2[:],
                                            axis=mybir.AxisListType.X,
                                            op=OP.max)
                    gap_t = pa.tile([128, 1], f32)
                    nc.vector.tensor_sub(gap_t[:], mxv[:], mx2[:])
                    # code = argmax + 128*(gap < tau), exact small ints;
                    # built from is_gt (known-good): am + 128 - 128*(gap>tau)
                    gt = pa.tile([128, 1], f32)
                    nc.vector.tensor_scalar(
                        out=gt[:], in0=gap_t[:], scalar1=tau_t[:],
                        scalar2=None, op0=OP.is_gt)
                    code_f = pa.tile([128, 1], f32)
                    nc.vector.scalar_tensor_tensor(
                        out=code_f[:], in0=gt[:], scalar=-128.0, in1=idxf[:],
                        op0=OP.mult, op1=OP.add)
                    nc.vector.tensor_scalar_add(code_f[:], code_f[:], 128.0)
                    code8 = pa.tile([128, 1], dt.uint8)
                    nc.vector.tensor_copy(code8[:], code_f[:])
                    nc.sync.dma_start(out=code_d[row0:row0 + 128, :],
                                      in_=code8[:])

    split_excess_waits(nc)
    return nc


def build(nb, nch, w, ghconv_dtype=dt.float32, nbu=None):
    """modM: full pipeline given host-computed sort ranks; bf16 x input."""
    NP = nch * BIN
    NBINS = nch
    if nbu is None:
        nbu = NBINS
    f32 = dt.float32
    bf16 = dt.bfloat16
    use_r = ghconv_dtype == dt.float32r
    gdt = ghconv_dtype

    nc = bass.Bass("TRN2", target_bir_lowering=False, debug=False)

    x_in = nc.dram_tensor("x", [nb * NP, F], bf16, kind="ExternalInput").ap()
    m_in = nc.dram_tensor("m", [nb * NP, 1], f32, kind="ExternalInput").ap()
    rank_in = nc.dram_tensor("rank", [nb * NP, 1], dt.uint32,
                             kind="ExternalInput").ap()
    wnames = ["W1g", "b1gb", "W2", "b2",
              "th0", "Wh0", "Wt0", "bth0", "bhh0", "bgt0",
              "th1", "Wh1", "Wt1", "bt1"]
    wdram = {n: nc.inline_tensor(w[n], name=n).ap() for n in wnames}
    outs = [nc.dram_tensor(f"out{b}", [nbu * BIN, F], dt.int8,
                           kind="ExternalOutput").ap()
            for b in range(nb)]
    osc = [nc.dram_tensor(f"osc{b}", [nbu * BIN, 1], f32,
                          kind="ExternalOutput").ap()
           for b in range(nb)]
    psort = [nc.dram_tensor(f"psort{b}", [NP, RW], f32, kind="Internal").ap()
             for b in range(nb)]

    with tile.TileContext(nc) as tc:
        with tc.tile_pool(name="init", bufs=1) as ip:
            ident = ip.tile([128, 128], f32)
            make_identity(nc, ident[:])
            eps_t = ip.tile([128, 1], f32)
            nc.vector.memset(eps_t[:], 1e-6)
            iota_p_i = ip.tile([128, 1], dt.int32)
            nc.gpsimd.iota(iota_p_i[:], [[0, 1]], base=0, channel_multiplier=1)
            iota_p_f = ip.tile([128, 1], f32)
            nc.vector.tensor_copy(iota_p_f[:], iota_p_i[:])
            ones_row_f = ip.tile([1, 128], f32)
            nc.vector.memset(ones_row_f[:], 1.0)
            ones_row_g = ip.tile([1, 128], gdt)
            if gdt == f32:
                nc.vector.memset(ones_row_g[:], 1.0)
            else:
                nc.vector.tensor_copy(ones_row_g[:], ones_row_f[:])

            # weights to SBUF
            wsb = {}
            for n in wnames:
                s = list(w[n].shape)
                wdt = f32
                if n in ("th0", "Wh0", "Wt0", "th1", "Wh1", "Wt1",
                         "bth0", "bhh0", "bgt0", "bt1"):
                    wdt = gdt
                shp = [128, s[0] // 128, s[1]] if s[0] > 128 else s
                src = (wdram[n].rearrange("(c p) m -> p c m", p=128)
                       if s[0] > 128 else wdram[n][:])
                if wdt == f32:
                    t = ip.tile(shp, f32, tag=f"w_{n}")
                    nc.gpsimd.dma_start(out=t[:], in_=src)
                else:
                    stg = ip.tile(shp, f32, tag="w_stage")
                    nc.gpsimd.dma_start(out=stg[:], in_=src)
                    t = ip.tile(shp, wdt, tag=f"w_{n}")
                    nc.vector.tensor_copy(t[:], stg[:])
                wsb[n] = t

            for b in range(nb):
                _one_batch(tc, nc, b, nb, nch, NP, NBINS, nbu,
                           x_in, m_in, rank_in, wsb, outs[b], osc[b],
                           psort[b], ident, eps_t, iota_p_f,
                           ones_row_f, ones_row_g, gdt, use_r)

    split_excess_waits(nc)
    return nc


def _one_batch(tc, nc, b, nb, nch, NP, NBINS, nbu,
               x_in, m_in, rank_in, wsb, out_d, osc_d, psort_d,
               ident, eps_t, iota_p_f, ones_row_f, ones_row_g, gdt, use_r):
    f32 = dt.float32
    bf16 = dt.bfloat16
    if use_r:
        def R(ap):
            return ap.bitcast(dt.float32r)
    else:
        def R(ap):
            return ap

    with tc.tile_pool(name=f"res{b}", bufs=1) as rp:
        packed = rp.tile([128, nch, RW], f32)     # resident z*m / xd / m / idx
        rank_u = rp.tile([128, nch], dt.uint32)

        # ---------------- phase A: LN -> ffn -> pack ----------------
        with tc.tile_pool(name=f"pa{b}", bufs=3) as pa, \
             tc.tile_pool(name=f"paps{b}", bufs=1, space="PSUM") as pap:
            for c in range(nch):
                row0 = b * NP + c * 128
                nc.sync.dma_start(out=packed[:, c, COL_M:COL_M + 1],
                                  in_=m_in[row0:row0 + 128, :])
                nc.sync.dma_start(out=rank_u[:, c:c + 1],
                                  in_=rank_in[row0:row0 + 128, :])
                m_ap = packed[:, c, COL_M:COL_M + 1]
                z_t, xdT_sb = _ffn_head(nc, pa, pap, wsb, ident, eps_t,
                                        ones_row_f, x_in[row0:row0 + 128, :])
                # zm into packed (gpsimd: SBUF only)
                nc.gpsimd.tensor_scalar_mul(packed[:, c, 0:F], z_t[:], m_ap)
                # xd point-major into packed
                xd_ps = pap.tile([128, 128], f32, space="PSUM")
                nc.tensor.transpose(xd_ps[:], xdT_sb[:], ident[:])
                nc.vector.tensor_copy(packed[:, c, F:F + 128], xd_ps[:])

        # ---------------- phase A2: scatter rows to sorted order ----------
        for c in range(nch):
            nc.gpsimd.indirect_dma_start(
                out=psort_d[:],
                out_offset=IOA(ap=rank_u[:, c:c + 1], axis=0),
                in_=packed[:, c, :], in_offset=None)

    # ---------------- phase B: adjacency + GHConv per bin ----------------
    if "noB" in ABLATION:
        return
    with tc.tile_pool(name=f"pb{b}", bufs=4) as pb, \
         tc.tile_pool(name=f"pbps{b}", bufs=1, space="PSUM") as pbp:
        for s in range(nbu):
            pk = pb.tile([128, RW], f32)
            nc.sync.dma_start(out=pk[:], in_=psort_d[s * 128:(s + 1) * 128, :])
            m_ap = pk[:, COL_M:COL_M + 1]
            # V cols: [na, one, one, na, m]; transposed pair/row tiles all
            # land at partition base 0 (matmul requires equal bases).
            V = pb.tile([128, 5], f32)
            sq = pb.tile([128, 128], f32)
            nc.scalar.activation(out=sq[:], in_=pk[:, F:F + 128],
                                 func=AF.Square, accum_out=V[:, 0:1])
            nc.gpsimd.memset(V[:, 1:3], 1.0)
            nc.gpsimd.tensor_copy(V[:, 3:4], V[:, 0:1])
            nc.gpsimd.tensor_copy(V[:, 4:5], m_ap)
            vt_ps = pbp.tile([2, 384], f32, space="PSUM")
            nc.tensor.transpose(vt_ps[0:2, 0:128], V[:, 0:2], ident[:])
            VTa = pb.tile([2, 128], f32)
            nc.scalar.activation(out=VTa[:], in_=vt_ps[0:2, 0:128],
                                 func=AF.Copy)
            nc.tensor.transpose(vt_ps[0:2, 128:256], V[:, 2:4], ident[:])
            VTb = pb.tile([2, 128], f32)
            nc.scalar.activation(out=VTb[:], in_=vt_ps[0:2, 128:256],
                                 func=AF.Copy)
            nc.tensor.transpose(vt_ps[0:1, 256:384], V[:, 4:5], ident[:])
            mT_sb = pb.tile([1, 128], f32)
            nc.scalar.activation(out=mT_sb[:], in_=vt_ps[0:1, 256:384],
                                 func=AF.Copy)
            # d2 = na_i - 2 xd xd^T + na_j ; M2 = m_i m_j
            adj_ps = pbp.tile([128, 384], f32, space="PSUM")
            xdT_ps = adj_ps[:, 0:128]
            d2_ps = adj_ps[:, 128:256]
            M2_ps = adj_ps[:, 256:384]
            nc.tensor.transpose(xdT_ps, pk[:, F:F + 128], ident[:])
            xdT = pb.tile([128, 128], f32)
            nc.scalar.activation(out=xdT[:], in_=xdT_ps, func=AF.Copy)
            xdTm2 = pb.tile([128, 128], f32)
            nc.scalar.activation(out=xdTm2[:], in_=xdT_ps, func=AF.Copy,
                                 scale=-2.0)
            nc.tensor.matmul(d2_ps, lhsT=xdTm2[:], rhs=xdT[:],
                             start=True, stop=False)
            nc.tensor.matmul(d2_ps, lhsT=VTa[:], rhs=VTb[:],
                             start=False, stop=True)
            nc.tensor.matmul(M2_ps, lhsT=mT_sb[:], rhs=mT_sb[:],
                             start=True, stop=True)
            dsc = pb.tile([128, 128], f32)
            nc.vector.tensor_scalar_max(dsc[:], d2_ps[:], 1e-6)
            nc.scalar.activation(out=dsc[:], in_=dsc[:], func=AF.Sqrt)
            nc.scalar.activation(out=dsc[:], in_=dsc[:], func=AF.Exp,
                                 scale=-0.1)
            dm = pb.tile([128, 128], gdt)
            ind = pb.tile([128, 1], f32)
            nc.vector.scalar_tensor_tensor(
                out=dm[:], in0=dsc[:], scalar=1.0, in1=M2_ps[:],
                op0=OP.mult, op1=OP.mult, accum_out=ind[:])
            nrm = pb.tile([128, 1], f32)
            nc.scalar.activation(out=nrm[:], in_=ind[:], func=AF.Sqrt,
                                 bias=eps_t[:])
            nc.vector.reciprocal(nrm[:], nrm[:])
            nc.vector.tensor_mul(nrm[:], nrm[:], m_ap)

            xb_ap = pk[:, 0:F]
            for li in range(2):
                sfx = "0" if li == 0 else "1"
                mm1 = pbp.tile([128, 512], f32, space="PSUM")
                mm2 = pbp.tile([128, 512], f32, space="PSUM")
                gat_ps = pbp.tile([128, F], f32, space="PSUM")
                xmT_ps = mm1[:, 0:256]
                hom2_ps = mm1[:, 256:512]
                hom_ps = mm2[:, 0:256]
                het_ps = mm2[:, 256:512]
                for k in range(2):
                    nc.tensor.transpose(
                        xmT_ps.rearrange("p (c q) -> p c q", q=128)[:, k, :],
                        xb_ap[:, k * 128:(k + 1) * 128], ident[:])
                xmT = pb.tile([128, 2, 128], gdt)
                nc.scalar.activation(out=xmT[:], in_=xmT_ps, func=AF.Copy)
                mT = mT_sb[:]
                if gdt != f32:
                    mTg = pb.tile([1, 128], gdt)
                    nc.vector.tensor_copy(mTg[:], mT_sb[:])
                    mT = mTg[:]
                # keep each PSUM accumulation group's matmuls consecutive
                for dst, wn, bias in (
                    (hom_ps, "th" + sfx, "bth0" if li == 0 else None),
                    (het_ps, "Wh" + sfx, "bhh0" if li == 0 else None),
                    (gat_ps[:], "Wt" + sfx,
                     "bgt0" if li == 0 else "bt1"),
                ):
                    for k in range(2):
                        nc.tensor.matmul(
                            dst, lhsT=R(xmT[:, k, :]), rhs=R(wsb[wn][:, k, :]),
                            start=(k == 0), stop=(k == 1 and bias is None))
                    if bias is not None:
                        blhs = mT if li == 0 else ones_row_g[:]
                        nc.tensor.matmul(dst, lhsT=R(blhs), rhs=R(wsb[bias][:]),
                                         start=False, stop=True)
                fh1 = pb.tile([128, F], gdt)
                nc.vector.tensor_scalar_mul(fh1[:], hom_ps[:], nrm[:])
                nc.tensor.matmul(hom2_ps[:], lhsT=R(dm[:]), rhs=R(fh1[:]),
                                 start=True, stop=True)
                gate = pb.tile([128, F], f32)
                nc.scalar.activation(out=gate[:], in_=gat_ps[:], func=AF.Sigmoid)
                fh2 = pb.tile([128, F], f32)
                nc.vector.tensor_scalar_mul(fh2[:], hom2_ps[:], nrm[:])
                nc.vector.tensor_sub(fh2[:], fh2[:], het_ps[:])
                nc.vector.tensor_mul(gate[:], gate[:], fh2[:])
                nc.vector.tensor_add(fh2[:], gate[:], het_ps[:])  # pre-act
                emin = pb.tile([128, F], f32)
                nc.gpsimd.tensor_scalar_min(emin[:], fh2[:], 0.0)
                nc.scalar.activation(out=emin[:], in_=emin[:], func=AF.Exp)
                er = pb.tile([128, F], f32)
                nc.scalar.activation(out=er[:], in_=fh2[:], func=AF.Relu)
                nc.vector.scalar_tensor_tensor(
                    out=emin[:], in0=emin[:], scalar=-1.0, in1=er[:],
                    op0=OP.add, op1=OP.add)
                out_t = pb.tile([128, F], f32)
                nc.gpsimd.tensor_scalar_mul(out_t[:], emin[:], m_ap)
                xb_ap = out_t[:]
            # emit sorted-order rows as int8 with a per-row fp32 scale
            # (rowmax/126.5 so the +0.5*sign rounding bias can never
            # saturate past 127) + original-index column; the host
            # dequantizes and scatters rows back to input order
            rabs = pb.tile([128, 1], f32)
            nc.vector.tensor_reduce(out=rabs[:], in_=xb_ap,
                                    axis=mybir.AxisListType.X, op=OP.max,
                                    apply_absolute_value=True)
            scq = pb.tile([128, 1], f32)
            nc.scalar.activation(out=scq[:], in_=rabs[:], func=AF.Copy,
                                 scale=1.0 / 126.5)
            nc.sync.dma_start(out=osc_d[s * 128:(s + 1) * 128, :], in_=scq[:])
            rc = pb.tile([128, 1], f32)
            nc.vector.tensor_scalar_max(rc[:], rabs[:], 1e-30)
            inv = pb.tile([128, 1], f32)
            nc.vector.reciprocal(inv[:], rc[:])
            inv127 = pb.tile([128, 1], f32)
            nc.scalar.activation(out=inv127[:], in_=inv[:], func=AF.Copy,
                                 scale=126.5)
            qf = pb.tile([128, F], f32)
            nc.vector.tensor_scalar_mul(qf[:], xb_ap, inv127[:])
            # round-to-nearest: add +-0.5 via (q>0)-0.5, then int convert
            sg = pb.tile([128, F], f32)
            nc.vector.tensor_scalar(
                out=sg[:], in0=qf[:], scalar1=0.0, scalar2=None, op0=OP.is_gt)
            nc.vector.scalar_tensor_tensor(
                out=qf[:], in0=sg[:], scalar=-0.5, in1=qf[:],
                op0=OP.add, op1=OP.add)
            q8 = pb.tile([128, F], dt.int8)
            nc.vector.tensor_copy(q8[:], qf[:])
            nc.sync.dma_start(out=out_d[s * 128:(s + 1) * 128, :], in_=q8[:])


def _fold_weights(inputs):
    g = inputs["ln_gamma"].astype(np.float32)
    be = inputs["ln_beta"].astype(np.float32)
    W1 = inputs["W1"].astype(np.float32)
    b1 = inputs["b1"].astype(np.float32)
    w = {
        "W1g": g[:, None] * W1,
        "b1gb": (b1 + be @ W1)[None, :],
        "W2": inputs["W2"].astype(np.float32),
        "b2": inputs["b2"].astype(np.float32)[None, :],
        "th1": inputs["th1"].astype(np.float32),
        "Wh1": inputs["Wh1"].astype(np.float32),
        "Wt1": inputs["Wt1"].astype(np.float32),
        "bt1": inputs["bt1"].astype(np.float32)[None, :],
    }
    for nm in ("th0", "Wh0", "Wt0"):
        w[nm] = g[:, None] * inputs[nm].astype(np.float32)
    w["bth0"] = (be @ inputs["th0"].astype(np.float32))[None, :]
    w["bhh0"] = (be @ inputs["Wh0"].astype(np.float32))[None, :]
    w["bgt0"] = (inputs["bt0"].astype(np.float32) +
                 be @ inputs["Wt0"].astype(np.float32))[None, :]
    return {k: np.ascontiguousarray(v, dtype=np.float32) for k, v in w.items()}


_RUNNER_CACHE = {}


def _make_runner(nc, n_cores):
    """Jit a Bass module for SPMD execution; returns the callable + metadata."""
    import jax
    from jax.sharding import Mesh, PartitionSpec, NamedSharding
    from jax.experimental.shard_map import shard_map
    from concourse import bass2jax

    partition_name = (nc.partition_id_tensor.name
                      if nc.partition_id_tensor else None)
    in_names, out_names, out_avals, zero_shapes = [], [], [], []
    for alloc in nc.m.functions[0].allocations:
        if not isinstance(alloc, mybir.MemoryLocationSet):
            continue
        name = alloc.memorylocations[0].name
        if alloc.kind == "ExternalInput":
            if name != partition_name:
                in_names.append(name)
        elif alloc.kind == "ExternalOutput":
            out_names.append(name)
            shape = tuple(alloc.tensor_shape)
            dtype = mybir.dt.np(alloc.dtype)
            out_avals.append(jax.core.ShapedArray(shape, dtype))
            zero_shapes.append((shape, dtype))
    n_params = len(in_names)
    all_names = in_names + out_names
    if partition_name is not None:
        all_names = all_names + [partition_name]

    def _body(*args):
        operands = list(args)
        if partition_name is not None:
            operands.append(bass2jax.partition_id_tensor())
        outs = bass2jax._bass_exec_p.bind(
            *operands,
            out_avals=tuple(out_avals),
            in_names=tuple(all_names),
            out_names=tuple(out_names),
            lowering_input_output_aliases=(),
            sim_require_finite=True,
            sim_require_nnan=True,
            nc=nc,
        )
        return tuple(outs)

    devices = jax.devices()[:n_cores]
    mesh = Mesh(np.asarray(devices), ("core",))
    in_specs = (PartitionSpec("core"),) * (n_params + len(out_names))
    out_specs = (PartitionSpec("core"),) * len(out_names)
    sharded = jax.jit(
        shard_map(_body, mesh=mesh, in_specs=in_specs, out_specs=out_specs,
                  check_rep=False),
        keep_unused=True)
    # zero output buffers staged on device ONCE and reused read-only
    shard = NamedSharding(mesh, PartitionSpec("core"))
    dev_zeros = [
        jax.device_put(np.zeros((n_cores * s0[0], *s0[1:]), d), shard)
        for s0, d in zero_shapes]
    return (sharded, in_names, out_names, out_avals, dev_zeros)


def _get_runners(nb, nch, ghconv_dtype, n_cores, nbu, w):
    """Cached (modK, modM) runners; weights are compile-time constants, so
    the cache key includes their fingerprint."""
    wkey = hashlib.blake2b(
        b"".join(w[k].tobytes() for k in sorted(w)), digest_size=16).hexdigest()
    key = (nb, nch, ghconv_dtype, n_cores, nbu, wkey, ABLATION)
    if key not in _RUNNER_CACHE:
        from concourse import bass2jax
        bass2jax.install_neuronx_cc_hook()
        ncK = build_keys(nb, nch, w, TAU)
        ncM = build(nb, nch, w, ghconv_dtype, nbu=nbu)
        _RUNNER_CACHE[key] = (_make_runner(ncK, n_cores),
                              _make_runner(ncM, n_cores))
    return _RUNNER_CACHE[key]


def _host_fix_keys(x2d, ridx, w, nbins):
    """Exact fp32 LSH argmax for the given row indices (matches the
    reference chain: LN (gamma/beta folded) -> ffn_dist -> argmax)."""
    xr = x2d[ridx].astype(np.float32)
    mu = xr.mean(-1, keepdims=True)
    var = ((xr - mu) ** 2).mean(-1, keepdims=True)
    zn = (xr - mu) / np.sqrt(var + 1e-6)
    h = zn @ w["W1g"] + w["b1gb"]
    h = np.where(h > 0, h, np.expm1(np.minimum(h, 0)))
    xd = h @ w["W2"] + w["b2"]
    mul = xd @ w["CB"]
    cmul = np.concatenate([mul, -mul], -1)
    return np.argmax(cmul, -1)


def run(inputs, nb, nch, n_cores, ghconv_dtype=dt.float32, trace=False,
        nbu=NBU):
    """inputs: dict with x [Btot, NP, F] float32, msk [Btot, NP] bool + weights.
    Btot must equal n_cores * nb."""
    import concurrent.futures as cf
    import jax
    from jax.sharding import Mesh, PartitionSpec, NamedSharding
    import ml_dtypes

    NP = nch * BIN
    NBINS = nch
    x = np.ascontiguousarray(inputs["x"], dtype=np.float32)
    msk = np.asarray(inputs["msk"])
    Btot = x.shape[0]
    assert Btot == n_cores * nb
    w = _fold_weights(inputs)
    w["CB"] = np.ascontiguousarray(
        inputs["codebook"][:, :NBINS // 2], dtype=np.float32)

    (rK, rM) = _get_runners(nb, nch, ghconv_dtype, n_cores, nbu, w)
    shardedK, in_namesK, out_namesK, _, dev_zerosK = rK
    shardedM, in_namesM, out_namesM, _, dev_zerosM = rM

    # ---- put: x once as bf16, shared by both dispatches ----
    x2d = x.reshape(Btot * NP, F)
    xb = x2d.astype(ml_dtypes.bfloat16)
    mf = msk.astype(np.float32).reshape(Btot * NP, 1)
    mesh = Mesh(np.asarray(jax.devices()[:n_cores]), ("core",))
    shard = NamedSharding(mesh, PartitionSpec("core"))
    xb_dev = jax.device_put(xb, shard)

    # ---- modK: LSH argmax + risky bit from bf16 x ----
    full = {"x": xb_dev}
    outK = shardedK(*[full[n] for n in in_namesK], *dev_zerosK)
    resK = dict(zip(out_namesK, outK))
    code = np.asarray(resK["code"]).reshape(Btot * NP)
    am = (code & 127).astype(np.int32)

    # ---- host: exact argmax for risky rows, then sort ranks ----
    ridx = np.nonzero(code >= 128)[0]
    if len(ridx):
        am[ridx] = _host_fix_keys(x2d, ridx, w, NBINS)
    keys = am.reshape(Btot, NP) + np.where(~msk, NBINS - 1, 0)
    perm = np.argsort(keys, axis=-1, kind="stable")
    ranks = np.empty((Btot, NP), np.uint32)
    ar = np.arange(NP, dtype=np.uint32)
    for bi in range(Btot):
        ranks[bi, perm[bi]] = ar
        # every unmasked row must sort into the emitted prefix
        if not (ranks[bi][msk[bi]] < nbu * BIN).all():
            raise RuntimeError(
                f"batch {bi}: unmasked rows beyond {nbu} sorted bins; "
                f"increase NBU")

    # ---- modM: main pipeline with exact ranks ----
    full = {"x": xb_dev, "m": mf, "rank": ranks.reshape(Btot * NP, 1)}
    outM = shardedM(*[full[n] for n in in_namesM], *dev_zerosM)
    resM = dict(zip(out_namesM, outM))

    # ---- fetch + dequantize + scatter back to input order; the dequant of
    # one output overlaps the fetch of the next ----
    out = np.zeros((Btot, NP, F), np.float32)
    with cf.ThreadPoolExecutor(2) as ex:
        fosc = {b: ex.submit(np.asarray, resM[f"osc{b}"]) for b in range(nb)}
        fout = {b: ex.submit(np.asarray, resM[f"out{b}"]) for b in range(nb)}
        for b in range(nb):
            q_all = fout[b].result().reshape(n_cores, nbu * BIN, F)
            sc_all = fosc[b].result().reshape(n_cores, nbu * BIN, 1)
            for core in range(n_cores):
                gb = core * nb + b
                ids = perm[gb, :nbu * BIN]
                out[gb, ids] = q_all[core].astype(np.float32) * sc_all[core]
    return out, None


def kernel(**inputs):
    out, _ = run(inputs, nb=2, nch=100, n_cores=8)
    return out


# revision 35
# speedup vs baseline: 1.5561x; 1.1129x over previous
"""Trainium2 Bass kernel for nn_CombinedGraphLayer (LSH-binned GHConv message passing).

Contract: kernel(**inputs) takes FULL inputs (x [16,12800,256], msk [16,12800],
training scalar + weights), returns FULL output [16,12800,256].

Strategy: pure data-parallel over batch (2 batches per NeuronCore x 8 cores).
The wall clock is dominated by the ~55-75 MB/s host<->device tunnel, so the
pipeline is organized around minimizing wire bytes:

  put   x as bf16 (105MB instead of 210MB fp32)
  modK  (device): LSH argmax + top-2 gap per row from the bf16 input
  host  rows whose gap < TAU could have a different argmax than the fp32
        reference chain; recompute those exactly in fp64-free numpy fp32
        (~10% of rows, ~0.2s), then argsort -> exact per-row sort ranks
  modM  (device): layernorm -> ffn_dist -> pack, indirect-scatter rows into
        sorted bin order using the host ranks, then per 128-point bin:
        pairwise gaussian adjacency + 2 GHConv layers. Only the first NBU
        sorted bins are computed/emitted (all unmasked rows sort there);
        output rows leave in sorted order as bf16 + original-index column
  host  scatter rows back to input order (bf16 output quantization adds
        ~2e-3 max-rel error vs the 2e-2 tolerance)

Weights are folded (layernorm gamma/beta into the ffn/GHConv weights) and
embedded in the NEFF as constants - zero per-call wire cost.
"""

import hashlib
import numpy as np

import concourse.bass as bass
import concourse.tile as tile
from concourse import mybir
from concourse.masks import make_identity

dt = mybir.dt
OP = mybir.AluOpType
AF = mybir.ActivationFunctionType
IOA = bass.IndirectOffsetOnAxis

ABLATION = ""  # bench knob: "noB" (timing experiments only)

F = 256       # feature dim
D = 128       # distance dim
BIN = 128

# packed row layout (fp32): [ zm(0:256) | xd(256:384) | m(384) | idx(385) | pad ]
RW = 388
COL_M = 384
COL_IDX = 385

NBU = 54   # sorted 128-row bins computed per batch; all unmasked rows land in
           # the first ~nch/2+1 bins (msk ~ Bernoulli(0.5)); runtime-verified.
TAU = 5e-3  # risky-gap threshold; max |cmul(bf16 x) - cmul(fp32 x)| measured
            # at 2.2e-3, so 5e-3 leaves >2x margin (zero non-risky flips seen
            # even at 4.3e-3). ~6% of rows get an exact host recompute of
            # their LSH argmax.


def split_excess_waits(nc):
    """This walrus build rejects instructions carrying more than a couple of
    sem waits (1 for CTRL-class like Drain, ~2 for compute). Move excess
    waits onto extra Drains inserted just before, on the same engine."""
    for f in nc.m.functions:
        for b in f.blocks:
            new_insts = []
            for inst in b.instructions:
                si = getattr(inst, "sync_info", None)
                ow = list(si.on_wait) if si is not None and si.on_wait else []
                limit = 1
                if len(ow) > limit and inst.engine is not None:
                    keep = ow[-limit:]
                    for w in ow[:-limit]:
                        d = mybir.InstNoOp(
                            name=nc.get_next_instruction_name(), ins=[], outs=[]
                        )
                        d.engine = inst.engine
                        d.sync_info = mybir.SyncInfo(on_wait=[w], on_update=[])
                        new_insts.append(d)
                    si.on_wait = keep
                new_insts.append(inst)
            b.instructions = new_insts


def _ffn_head(nc, pa, pap, wsb, ident, eps_t, ones_row_f, xb_chunk, m_chunk=None):
    """Shared LN -> ffn_dist chain for one 128-row chunk of bf16 input.
    Returns (z_t fp32 [128,F], xdT_sb fp32 [128,128] feature-major)."""
    f32 = dt.float32
    xb_t = pa.tile([128, F], dt.bfloat16)
    nc.sync.dma_start(out=xb_t[:], in_=xb_chunk)
    x_t = pa.tile([128, F], f32)
    nc.vector.tensor_copy(x_t[:], xb_t[:])

    st = pa.tile([128, 6], f32)
    nc.vector.bn_stats(out=st[:], in_=x_t[:])
    mv = pa.tile([128, 2], f32)
    nc.vector.bn_aggr(out=mv[:], in_=st[:])
    nc.scalar.activation(out=mv[:, 1:2], in_=mv[:, 1:2],
                         func=AF.Sqrt, bias=eps_t[:])
    nc.vector.reciprocal(out=mv[:, 1:2], in_=mv[:, 1:2])
    z_t = pa.tile([128, F], f32)
    nc.vector.tensor_scalar(
        out=z_t[:], in0=x_t[:], scalar1=mv[:, 0:1],
        scalar2=mv[:, 1:2], op0=OP.subtract, op1=OP.mult)

    # zT (feature-major) for the ffn matmuls
    zT_ps = pap.tile([128, 2, 128], f32, space="PSUM")
    for k in range(2):
        nc.tensor.transpose(zT_ps[:, k, :],
                            z_t[:, k * 128:(k + 1) * 128], ident[:])
    zT_sb = pa.tile([128, 2, 128], f32)
    nc.scalar.activation(out=zT_sb[:], in_=zT_ps[:], func=AF.Copy)

    # hT = W1g^T zT + b1gb  (feature-major [D, pts])
    h_ps = pap.tile([128, 128], f32, space="PSUM")
    nc.tensor.matmul(h_ps[:], lhsT=wsb["W1g"][:, 0, :],
                     rhs=zT_sb[:, 0, :], start=True, stop=False)
    nc.tensor.matmul(h_ps[:], lhsT=wsb["W1g"][:, 1, :],
                     rhs=zT_sb[:, 1, :], start=False, stop=False)
    nc.tensor.matmul(h_ps[:], lhsT=wsb["b1gb"][:],
                     rhs=ones_row_f[:], start=False, stop=True)
    # elu
    e_t = pa.tile([128, 128], f32)
    nc.vector.tensor_scalar_min(e_t[:], h_ps[:], 0.0)
    nc.scalar.activation(out=e_t[:], in_=e_t[:], func=AF.Exp)
    r_t = pa.tile([128, 128], f32)
    nc.scalar.activation(out=r_t[:], in_=h_ps[:], func=AF.Relu)
    hTe = pa.tile([128, 128], f32)
    nc.vector.scalar_tensor_tensor(
        out=hTe[:], in0=e_t[:], scalar=-1.0, in1=r_t[:],
        op0=OP.add, op1=OP.add)

    # xdT = W2^T hTe + b2
    xdT_ps = pap.tile([128, 128], f32, space="PSUM")
    nc.tensor.matmul(xdT_ps[:], lhsT=wsb["W2"][:], rhs=hTe[:],
                     start=True, stop=False)
    nc.tensor.matmul(xdT_ps[:], lhsT=wsb["b2"][:],
                     rhs=ones_row_f[:], start=False, stop=True)
    xdT_sb = pa.tile([128, 128], f32)
    nc.scalar.activation(out=xdT_sb[:], in_=xdT_ps[:], func=AF.Copy)
    return z_t, xdT_sb


def build_keys(nb, nch, w, tau):
    """modK: per-row LSH (argmax + 128*risky) packed as one uint8 output;
    risky = top-2 gap below tau."""
    NP = nch * BIN
    CB = nch // 2
    f32 = dt.float32
    nc = bass.Bass("TRN2", target_bir_lowering=False, debug=False)

    x_in = nc.dram_tensor("x", [nb * NP, F], dt.bfloat16,
                          kind="ExternalInput").ap()
    code_d = nc.dram_tensor("code", [nb * NP, 1], dt.uint8,
                            kind="ExternalOutput").ap()
    wdram = {n: nc.inline_tensor(w[n], name=n).ap()
             for n in ("W1g", "b1gb", "W2", "b2", "CB")}

    with tile.TileContext(nc) as tc:
        with tc.tile_pool(name="init", bufs=1) as ip:
            ident = ip.tile([128, 128], f32)
            make_identity(nc, ident[:])
            eps_t = ip.tile([128, 1], f32)
            nc.vector.memset(eps_t[:], 1e-6)
            ones_row_f = ip.tile([1, 128], f32)
            nc.vector.memset(ones_row_f[:], 1.0)
            tau_t = ip.tile([128, 1], f32)
            nc.vector.memset(tau_t[:], tau)
            wsb = {}
            for n in ("W1g", "b1gb", "W2", "b2", "CB"):
                s = list(w[n].shape)
                shp = [128, s[0] // 128, s[1]] if s[0] > 128 else s
                src = (wdram[n].rearrange("(c p) m -> p c m", p=128)
                       if s[0] > 128 else wdram[n][:])
                t = ip.tile(shp, f32, tag=f"w_{n}")
                nc.gpsimd.dma_start(out=t[:], in_=src)
                wsb[n] = t

            with tc.tile_pool(name="pk", bufs=3) as pa, \
                 tc.tile_pool(name="pkps", bufs=1, space="PSUM") as pap:
                for c in range(nb * nch):
                    row0 = c * 128
                    _, xdT_sb = _ffn_head(nc, pa, pap, wsb, ident, eps_t,
                                          ones_row_f,
                                          x_in[row0:row0 + 128, :])
                    # mul = xd @ codebook  (point-major [pts, CB])
                    mul_ps = pap.tile([128, CB], f32, space="PSUM")
                    nc.tensor.matmul(mul_ps[:], lhsT=xdT_sb[:], rhs=wsb["CB"][:],
                                     start=True, stop=True)
                    cmul = pa.tile([128, 2 * CB], f32)
                    nc.scalar.activation(out=cmul[:, 0:CB], in_=mul_ps[:],
                                         func=AF.Copy)
                    nc.scalar.activation(out=cmul[:, CB:2 * CB], in_=mul_ps[:],
                                         func=AF.Copy, scale=-1.0)
                    mx8 = pa.tile([128, 8], f32)
                    nc.vector.max(out=mx8[:], in_=cmul[:])
                    ix8 = pa.tile([128, 8], dt.uint32)
                    nc.vector.max_index(out=ix8[:], in_max=mx8[:],
                                        in_values=cmul[:])
                    idxf = pa.tile([128, 1], f32)
                    nc.vector.tensor_copy(idxf[:], ix8[:, 0:1])
                    # top-2 gap: mask out max positions, re-reduce
                    mxv = pa.tile([128, 1], f32)
                    nc.vector.tensor_reduce(out=mxv[:], in_=cmul[:],
                                            axis=mybir.AxisListType.X,
                                            op=OP.max)
                    eq = pa.tile([128, 2 * CB], f32)
                    nc.vector.tensor_scalar(
                        out=eq[:], in0=cmul[:], scalar1=mxv[:],
                        scalar2=None, op0=OP.is_equal)
                    c2 = pa.tile([128, 2 * CB], f32)
                    nc.vector.scalar_tensor_tensor(
                        out=c2[:], in0=eq[:], scalar=-1e30, in1=cmul[:],
                        op0=OP.mult, op1=OP.add)
                    mx2 = pa.tile([128, 1], f32)
                    nc.vector.tensor_reduce(out=mx2[:], in_=c2[:],
                                            axis=mybir.AxisListType.X,
                                            op=OP.max)
                    gap_t = pa.tile([128, 1], f32)
                    nc.vector.tensor_sub(gap_t[:], mxv[:], mx2[:])
                    # code = argmax + 128*(gap < tau), exact small ints;
                    # built from is_gt (known-good): am + 128 - 128*(gap>tau)
                    gt = pa.tile([128, 1], f32)
                    nc.vector.tensor_scalar(
                        out=gt[:], in0=gap_t[:], scalar1=tau_t[:],
                        scalar2=None, op0=OP.is_gt)
                    code_f = pa.tile([128, 1], f32)
                    nc.vector.scalar_tensor_tensor(
                        out=code_f[:], in0=gt[:], scalar=-128.0, in1=idxf[:],
                        op0=OP.mult, op1=OP.add)
                    nc.vector.tensor_scalar_add(code_f[:], code_f[:], 128.0)
                    code8 = pa.tile([128, 1], dt.uint8)
                    nc.vector.tensor_copy(code8[:], code_f[:])
                    nc.sync.dma_start(out=code_d[row0:row0 + 128, :],
                                      in_=code8[:])

    split_excess_waits(nc)
    return nc


def build(nb, nch, w, ghconv_dtype=dt.float32, nbu=None):
    """modM: full pipeline given host-computed sort ranks; bf16 x input."""
    NP = nch * BIN
    NBINS = nch
    if nbu is None:
        nbu = NBINS
    f32 = dt.float32
    bf16 = dt.bfloat16
    use_r = ghconv_dtype == dt.float32r
    gdt = ghconv_dtype

    nc = bass.Bass("TRN2", target_bir_lowering=False, debug=False)

    xs_in = [nc.dram_tensor(f"x{b}", [NP, F], bf16, kind="ExternalInput").ap()
             for b in range(nb)]
    m_in = nc.dram_tensor("m", [nb * NP, 1], f32, kind="ExternalInput").ap()
    rank_in = nc.dram_tensor("rank", [nb * NP, 1], dt.uint32,
                             kind="ExternalInput").ap()
    wnames = ["W1g", "b1gb", "W2", "b2",
              "th0", "Wh0", "Wt0", "bth0", "bhh0", "bgt0",
              "th1", "Wh1", "Wt1", "bt1"]
    wdram = {n: nc.inline_tensor(w[n], name=n).ap() for n in wnames}
    outs = [nc.dram_tensor(f"out{b}", [nbu * BIN, F], dt.int8,
                           kind="ExternalOutput").ap()
            for b in range(nb)]
    osc = [nc.dram_tensor(f"osc{b}", [nbu * BIN, 1], f32,
                          kind="ExternalOutput").ap()
           for b in range(nb)]
    psort = [nc.dram_tensor(f"psort{b}", [NP, RW], f32, kind="Internal").ap()
             for b in range(nb)]

    with tile.TileContext(nc) as tc:
        with tc.tile_pool(name="init", bufs=1) as ip:
            ident = ip.tile([128, 128], f32)
            make_identity(nc, ident[:])
            eps_t = ip.tile([128, 1], f32)
            nc.vector.memset(eps_t[:], 1e-6)
            iota_p_i = ip.tile([128, 1], dt.int32)
            nc.gpsimd.iota(iota_p_i[:], [[0, 1]], base=0, channel_multiplier=1)
            iota_p_f = ip.tile([128, 1], f32)
            nc.vector.tensor_copy(iota_p_f[:], iota_p_i[:])
            ones_row_f = ip.tile([1, 128], f32)
            nc.vector.memset(ones_row_f[:], 1.0)
            ones_row_g = ip.tile([1, 128], gdt)
            if gdt == f32:
                nc.vector.memset(ones_row_g[:], 1.0)
            else:
                nc.vector.tensor_copy(ones_row_g[:], ones_row_f[:])

            # weights to SBUF
            wsb = {}
            for n in wnames:
                s = list(w[n].shape)
                wdt = f32
                if n in ("th0", "Wh0", "Wt0", "th1", "Wh1", "Wt1",
                         "bth0", "bhh0", "bgt0", "bt1"):
                    wdt = gdt
                shp = [128, s[0] // 128, s[1]] if s[0] > 128 else s
                src = (wdram[n].rearrange("(c p) m -> p c m", p=128)
                       if s[0] > 128 else wdram[n][:])
                if wdt == f32:
                    t = ip.tile(shp, f32, tag=f"w_{n}")
                    nc.gpsimd.dma_start(out=t[:], in_=src)
                else:
                    stg = ip.tile(shp, f32, tag="w_stage")
                    nc.gpsimd.dma_start(out=stg[:], in_=src)
                    t = ip.tile(shp, wdt, tag=f"w_{n}")
                    nc.vector.tensor_copy(t[:], stg[:])
                wsb[n] = t

            for b in range(nb):
                _one_batch(tc, nc, b, nb, nch, NP, NBINS, nbu,
                           xs_in[b], m_in, rank_in, wsb, outs[b], osc[b],
                           psort[b], ident, eps_t, iota_p_f,
                           ones_row_f, ones_row_g, gdt, use_r)

    split_excess_waits(nc)
    return nc


def _one_batch(tc, nc, b, nb, nch, NP, NBINS, nbu,
               x_in, m_in, rank_in, wsb, out_d, osc_d, psort_d,
               ident, eps_t, iota_p_f, ones_row_f, ones_row_g, gdt, use_r):
    f32 = dt.float32
    bf16 = dt.bfloat16
    if use_r:
        def R(ap):
            return ap.bitcast(dt.float32r)
    else:
        def R(ap):
            return ap

    with tc.tile_pool(name=f"res{b}", bufs=1) as rp:
        packed = rp.tile([128, nch, RW], f32)     # resident z*m / xd / m / idx
        rank_u = rp.tile([128, nch], dt.uint32)

        # ---------------- phase A: LN -> ffn -> pack ----------------
        with tc.tile_pool(name=f"pa{b}", bufs=3) as pa, \
             tc.tile_pool(name=f"paps{b}", bufs=1, space="PSUM") as pap:
            for c in range(nch):
                row0 = b * NP + c * 128
                nc.sync.dma_start(out=packed[:, c, COL_M:COL_M + 1],
                                  in_=m_in[row0:row0 + 128, :])
                nc.sync.dma_start(out=rank_u[:, c:c + 1],
                                  in_=rank_in[row0:row0 + 128, :])
                m_ap = packed[:, c, COL_M:COL_M + 1]
                z_t, xdT_sb = _ffn_head(
                    nc, pa, pap, wsb, ident, eps_t, ones_row_f,
                    x_in[c * 128:(c + 1) * 128, :])
                # zm into packed (gpsimd: SBUF only)
                nc.gpsimd.tensor_scalar_mul(packed[:, c, 0:F], z_t[:], m_ap)
                # xd point-major into packed
                xd_ps = pap.tile([128, 128], f32, space="PSUM")
                nc.tensor.transpose(xd_ps[:], xdT_sb[:], ident[:])
                nc.vector.tensor_copy(packed[:, c, F:F + 128], xd_ps[:])

        # ---------------- phase A2: scatter rows to sorted order ----------
        for c in range(nch):
            nc.gpsimd.indirect_dma_start(
                out=psort_d[:],
                out_offset=IOA(ap=rank_u[:, c:c + 1], axis=0),
                in_=packed[:, c, :], in_offset=None)

    # ---------------- phase B: adjacency + GHConv per bin ----------------
    if "noB" in ABLATION:
        return
    with tc.tile_pool(name=f"pb{b}", bufs=4) as pb, \
         tc.tile_pool(name=f"pbps{b}", bufs=1, space="PSUM") as pbp:
        for s in range(nbu):
            pk = pb.tile([128, RW], f32)
            nc.sync.dma_start(out=pk[:], in_=psort_d[s * 128:(s + 1) * 128, :])
            m_ap = pk[:, COL_M:COL_M + 1]
            # V cols: [na, one, one, na, m]; transposed pair/row tiles all
            # land at partition base 0 (matmul requires equal bases).
            V = pb.tile([128, 5], f32)
            sq = pb.tile([128, 128], f32)
            nc.scalar.activation(out=sq[:], in_=pk[:, F:F + 128],
                                 func=AF.Square, accum_out=V[:, 0:1])
            nc.gpsimd.memset(V[:, 1:3], 1.0)
            nc.gpsimd.tensor_copy(V[:, 3:4], V[:, 0:1])
            nc.gpsimd.tensor_copy(V[:, 4:5], m_ap)
            vt_ps = pbp.tile([2, 384], f32, space="PSUM")
            nc.tensor.transpose(vt_ps[0:2, 0:128], V[:, 0:2], ident[:])
            VTa = pb.tile([2, 128], f32)
            nc.scalar.activation(out=VTa[:], in_=vt_ps[0:2, 0:128],
                                 func=AF.Copy)
            nc.tensor.transpose(vt_ps[0:2, 128:256], V[:, 2:4], ident[:])
            VTb = pb.tile([2, 128], f32)
            nc.scalar.activation(out=VTb[:], in_=vt_ps[0:2, 128:256],
                                 func=AF.Copy)
            nc.tensor.transpose(vt_ps[0:1, 256:384], V[:, 4:5], ident[:])
            mT_sb = pb.tile([1, 128], f32)
            nc.scalar.activation(out=mT_sb[:], in_=vt_ps[0:1, 256:384],
                                 func=AF.Copy)
            # d2 = na_i - 2 xd xd^T + na_j ; M2 = m_i m_j
            adj_ps = pbp.tile([128, 384], f32, space="PSUM")
            xdT_ps = adj_ps[:, 0:128]
            d2_ps = adj_ps[:, 128:256]
            M2_ps = adj_ps[:, 256:384]
            nc.tensor.transpose(xdT_ps, pk[:, F:F + 128], ident[:])
            xdT = pb.tile([128, 128], f32)
            nc.scalar.activation(out=xdT[:], in_=xdT_ps, func=AF.Copy)
            xdTm2 = pb.tile([128, 128], f32)
            nc.scalar.activation(out=xdTm2[:], in_=xdT_ps, func=AF.Copy,
                                 scale=-2.0)
            nc.tensor.matmul(d2_ps, lhsT=xdTm2[:], rhs=xdT[:],
                             start=True, stop=False)
            nc.tensor.matmul(d2_ps, lhsT=VTa[:], rhs=VTb[:],
                             start=False, stop=True)
            nc.tensor.matmul(M2_ps, lhsT=mT_sb[:], rhs=mT_sb[:],
                             start=True, stop=True)
            dsc = pb.tile([128, 128], f32)
            nc.vector.tensor_scalar_max(dsc[:], d2_ps[:], 1e-6)
            nc.scalar.activation(out=dsc[:], in_=dsc[:], func=AF.Sqrt)
            nc.scalar.activation(out=dsc[:], in_=dsc[:], func=AF.Exp,
                                 scale=-0.1)
            dm = pb.tile([128, 128], gdt)
            ind = pb.tile([128, 1], f32)
            nc.vector.scalar_tensor_tensor(
                out=dm[:], in0=dsc[:], scalar=1.0, in1=M2_ps[:],
                op0=OP.mult, op1=OP.mult, accum_out=ind[:])
            nrm = pb.tile([128, 1], f32)
            nc.scalar.activation(out=nrm[:], in_=ind[:], func=AF.Sqrt,
                                 bias=eps_t[:])
            nc.vector.reciprocal(nrm[:], nrm[:])
            nc.vector.tensor_mul(nrm[:], nrm[:], m_ap)

            xb_ap = pk[:, 0:F]
            for li in range(2):
                sfx = "0" if li == 0 else "1"
                mm1 = pbp.tile([128, 512], f32, space="PSUM")
                mm2 = pbp.tile([128, 512], f32, space="PSUM")
                gat_ps = pbp.tile([128, F], f32, space="PSUM")
                xmT_ps = mm1[:, 0:256]
                hom2_ps = mm1[:, 256:512]
                hom_ps = mm2[:, 0:256]
                het_ps = mm2[:, 256:512]
                for k in range(2):
                    nc.tensor.transpose(
                        xmT_ps.rearrange("p (c q) -> p c q", q=128)[:, k, :],
                        xb_ap[:, k * 128:(k + 1) * 128], ident[:])
                xmT = pb.tile([128, 2, 128], gdt)
                nc.scalar.activation(out=xmT[:], in_=xmT_ps, func=AF.Copy)
                mT = mT_sb[:]
                if gdt != f32:
                    mTg = pb.tile([1, 128], gdt)
                    nc.vector.tensor_copy(mTg[:], mT_sb[:])
                    mT = mTg[:]
                # keep each PSUM accumulation group's matmuls consecutive
                for dst, wn, bias in (
                    (hom_ps, "th" + sfx, "bth0" if li == 0 else None),
                    (het_ps, "Wh" + sfx, "bhh0" if li == 0 else None),
                    (gat_ps[:], "Wt" + sfx,
                     "bgt0" if li == 0 else "bt1"),
                ):
                    for k in range(2):
                        nc.tensor.matmul(
                            dst, lhsT=R(xmT[:, k, :]), rhs=R(wsb[wn][:, k, :]),
                            start=(k == 0), stop=(k == 1 and bias is None))
                    if bias is not None:
                        blhs = mT if li == 0 else ones_row_g[:]
                        nc.tensor.matmul(dst, lhsT=R(blhs), rhs=R(wsb[bias][:]),
                                         start=False, stop=True)
                fh1 = pb.tile([128, F], gdt)
                nc.vector.tensor_scalar_mul(fh1[:], hom_ps[:], nrm[:])
                nc.tensor.matmul(hom2_ps[:], lhsT=R(dm[:]), rhs=R(fh1[:]),
                                 start=True, stop=True)
                gate = pb.tile([128, F], f32)
                nc.scalar.activation(out=gate[:], in_=gat_ps[:], func=AF.Sigmoid)
                fh2 = pb.tile([128, F], f32)
                nc.vector.tensor_scalar_mul(fh2[:], hom2_ps[:], nrm[:])
                nc.vector.tensor_sub(fh2[:], fh2[:], het_ps[:])
                nc.vector.tensor_mul(gate[:], gate[:], fh2[:])
                nc.vector.tensor_add(fh2[:], gate[:], het_ps[:])  # pre-act
                emin = pb.tile([128, F], f32)
                nc.gpsimd.tensor_scalar_min(emin[:], fh2[:], 0.0)
                nc.scalar.activation(out=emin[:], in_=emin[:], func=AF.Exp)
                er = pb.tile([128, F], f32)
                nc.scalar.activation(out=er[:], in_=fh2[:], func=AF.Relu)
                nc.vector.scalar_tensor_tensor(
                    out=emin[:], in0=emin[:], scalar=-1.0, in1=er[:],
                    op0=OP.add, op1=OP.add)
                out_t = pb.tile([128, F], f32)
                nc.gpsimd.tensor_scalar_mul(out_t[:], emin[:], m_ap)
                xb_ap = out_t[:]
            # emit sorted-order rows as int8 with a per-row fp32 scale
            # (rowmax/126.5 so the +0.5*sign rounding bias can never
            # saturate past 127) + original-index column; the host
            # dequantizes and scatters rows back to input order
            rabs = pb.tile([128, 1], f32)
            nc.vector.tensor_reduce(out=rabs[:], in_=xb_ap,
                                    axis=mybir.AxisListType.X, op=OP.max,
                                    apply_absolute_value=True)
            scq = pb.tile([128, 1], f32)
            nc.scalar.activation(out=scq[:], in_=rabs[:], func=AF.Copy,
                                 scale=1.0 / 126.5)
            nc.sync.dma_start(out=osc_d[s * 128:(s + 1) * 128, :], in_=scq[:])
            rc = pb.tile([128, 1], f32)
            nc.vector.tensor_scalar_max(rc[:], rabs[:], 1e-30)
            inv = pb.tile([128, 1], f32)
            nc.vector.reciprocal(inv[:], rc[:])
            inv127 = pb.tile([128, 1], f32)
            nc.scalar.activation(out=inv127[:], in_=inv[:], func=AF.Copy,
                                 scale=126.5)
            qf = pb.tile([128, F], f32)
            nc.vector.tensor_scalar_mul(qf[:], xb_ap, inv127[:])
            # round-to-nearest: add +-0.5 via (q>0)-0.5, then int convert
            sg = pb.tile([128, F], f32)
            nc.vector.tensor_scalar(
                out=sg[:], in0=qf[:], scalar1=0.0, scalar2=None, op0=OP.is_gt)
            nc.vector.scalar_tensor_tensor(
                out=qf[:], in0=sg[:], scalar=-0.5, in1=qf[:],
                op0=OP.add, op1=OP.add)
            q8 = pb.tile([128, F], dt.int8)
            nc.vector.tensor_copy(q8[:], qf[:])
            nc.sync.dma_start(out=out_d[s * 128:(s + 1) * 128, :], in_=q8[:])


def _fold_weights(inputs):
    g = inputs["ln_gamma"].astype(np.float32)
    be = inputs["ln_beta"].astype(np.float32)
    W1 = inputs["W1"].astype(np.float32)
    b1 = inputs["b1"].astype(np.float32)
    w = {
        "W1g": g[:, None] * W1,
        "b1gb": (b1 + be @ W1)[None, :],
        "W2": inputs["W2"].astype(np.float32),
        "b2": inputs["b2"].astype(np.float32)[None, :],
        "th1": inputs["th1"].astype(np.float32),
        "Wh1": inputs["Wh1"].astype(np.float32),
        "Wt1": inputs["Wt1"].astype(np.float32),
        "bt1": inputs["bt1"].astype(np.float32)[None, :],
    }
    for nm in ("th0", "Wh0", "Wt0"):
        w[nm] = g[:, None] * inputs[nm].astype(np.float32)
    w["bth0"] = (be @ inputs["th0"].astype(np.float32))[None, :]
    w["bhh0"] = (be @ inputs["Wh0"].astype(np.float32))[None, :]
    w["bgt0"] = (inputs["bt0"].astype(np.float32) +
                 be @ inputs["Wt0"].astype(np.float32))[None, :]
    return {k: np.ascontiguousarray(v, dtype=np.float32) for k, v in w.items()}


_RUNNER_CACHE = {}


def _make_runner(nc, n_cores):
    """Jit a Bass module for SPMD execution; returns the callable + metadata."""
    import jax
    from jax.sharding import Mesh, PartitionSpec, NamedSharding
    from jax.experimental.shard_map import shard_map
    from concourse import bass2jax

    partition_name = (nc.partition_id_tensor.name
                      if nc.partition_id_tensor else None)
    in_names, out_names, out_avals, zero_shapes = [], [], [], []
    for alloc in nc.m.functions[0].allocations:
        if not isinstance(alloc, mybir.MemoryLocationSet):
            continue
        name = alloc.memorylocations[0].name
        if alloc.kind == "ExternalInput":
            if name != partition_name:
                in_names.append(name)
        elif alloc.kind == "ExternalOutput":
            out_names.append(name)
            shape = tuple(alloc.tensor_shape)
            dtype = mybir.dt.np(alloc.dtype)
            out_avals.append(jax.core.ShapedArray(shape, dtype))
            zero_shapes.append((shape, dtype))
    n_params = len(in_names)
    all_names = in_names + out_names
    if partition_name is not None:
        all_names = all_names + [partition_name]

    def _body(*args):
        operands = list(args)
        if partition_name is not None:
            operands.append(bass2jax.partition_id_tensor())
        outs = bass2jax._bass_exec_p.bind(
            *operands,
            out_avals=tuple(out_avals),
            in_names=tuple(all_names),
            out_names=tuple(out_names),
            lowering_input_output_aliases=(),
            sim_require_finite=True,
            sim_require_nnan=True,
            nc=nc,
        )
        return tuple(outs)

    devices = jax.devices()[:n_cores]
    mesh = Mesh(np.asarray(devices), ("core",))
    in_specs = (PartitionSpec("core"),) * (n_params + len(out_names))
    out_specs = (PartitionSpec("core"),) * len(out_names)
    sharded = jax.jit(
        shard_map(_body, mesh=mesh, in_specs=in_specs, out_specs=out_specs,
                  check_rep=False),
        keep_unused=True)
    # zero output buffers staged on device ONCE and reused read-only
    shard = NamedSharding(mesh, PartitionSpec("core"))
    dev_zeros = [
        jax.device_put(np.zeros((n_cores * s0[0], *s0[1:]), d), shard)
        for s0, d in zero_shapes]
    return (sharded, in_names, out_names, out_avals, dev_zeros)


def _get_runners(nb, nch, ghconv_dtype, n_cores, nbu, w):
    """Cached (modK, modM) runners; weights are compile-time constants, so
    the cache key includes their fingerprint."""
    wkey = hashlib.blake2b(
        b"".join(w[k].tobytes() for k in sorted(w)), digest_size=16).hexdigest()
    key = (nb, nch, ghconv_dtype, n_cores, nbu, wkey, ABLATION)
    if key not in _RUNNER_CACHE:
        from concourse import bass2jax
        bass2jax.install_neuronx_cc_hook()
        # modK handles ONE batch per core per dispatch so its input put,
        # execution, and the host fixup pipeline across the nb batches
        ncK = build_keys(1, nch, w, TAU)
        ncM = build(nb, nch, w, ghconv_dtype, nbu=nbu)
        _RUNNER_CACHE[key] = (_make_runner(ncK, n_cores),
                              _make_runner(ncM, n_cores))
    return _RUNNER_CACHE[key]


def _host_fix_keys(x2d, ridx, w, nbins):
    """Exact fp32 LSH argmax for the given row indices (matches the
    reference chain: LN (gamma/beta folded) -> ffn_dist -> argmax)."""
    xr = x2d[ridx].astype(np.float32)
    mu = xr.mean(-1, keepdims=True)
    var = ((xr - mu) ** 2).mean(-1, keepdims=True)
    zn = (xr - mu) / np.sqrt(var + 1e-6)
    h = zn @ w["W1g"] + w["b1gb"]
    h = np.where(h > 0, h, np.expm1(np.minimum(h, 0)))
    xd = h @ w["W2"] + w["b2"]
    mul = xd @ w["CB"]
    cmul = np.concatenate([mul, -mul], -1)
    return np.argmax(cmul, -1)


def run(inputs, nb, nch, n_cores, ghconv_dtype=dt.float32, trace=False,
        nbu=NBU):
    """inputs: dict with x [Btot, NP, F] float32, msk [Btot, NP] bool + weights.
    Btot must equal n_cores * nb."""
    import concurrent.futures as cf
    import jax
    from jax.sharding import Mesh, PartitionSpec, NamedSharding
    import ml_dtypes

    NP = nch * BIN
    NBINS = nch
    x = np.ascontiguousarray(inputs["x"], dtype=np.float32)
    msk = np.asarray(inputs["msk"])
    Btot = x.shape[0]
    assert Btot == n_cores * nb
    w = _fold_weights(inputs)
    w["CB"] = np.ascontiguousarray(
        inputs["codebook"][:, :NBINS // 2], dtype=np.float32)

    (rK, rM) = _get_runners(nb, nch, ghconv_dtype, n_cores, nbu, w)
    shardedK, in_namesK, out_namesK, _, dev_zerosK = rK
    shardedM, in_namesM, out_namesM, _, dev_zerosM = rM

    # ---- put: x as bf16 in nb batch-slices, casts and key fixups overlap
    # the async transfers ----
    x2d = x.reshape(Btot * NP, F)
    mesh = Mesh(np.asarray(jax.devices()[:n_cores]), ("core",))
    shard = NamedSharding(mesh, PartitionSpec("core"))
    xb_dev, outKs = [], []
    for h in range(nb):
        # batch h of every core: global batches h, nb+h, 2nb+h, ...
        xh = x[h::nb].astype(ml_dtypes.bfloat16).reshape(n_cores * NP, F)
        xb_dev.append(jax.device_put(xh, shard))
        outKs.append(shardedK(xb_dev[h], *dev_zerosK))
    mf = msk.astype(np.float32).reshape(Btot * NP, 1)

    # ---- fetch modK codes per slice; exact argmax for risky rows ----
    am = np.empty((Btot, NP), np.int32)
    for h in range(nb):
        code = np.asarray(outKs[h][0]).reshape(n_cores * NP)
        amh = (code & 127).astype(np.int32)
        ridx = np.nonzero(code >= 128)[0]
        if len(ridx):
            # map slice rows back to rows of the full x
            gidx = ((ridx // NP) * nb + h) * NP + ridx % NP
            amh[ridx] = _host_fix_keys(x2d, gidx, w, NBINS)
        am[h::nb] = amh.reshape(n_cores, NP)

    keys = am + np.where(~msk, NBINS - 1, 0)
    perm = np.argsort(keys, axis=-1, kind="stable")
    ranks = np.empty((Btot, NP), np.uint32)
    ar = np.arange(NP, dtype=np.uint32)
    for bi in range(Btot):
        ranks[bi, perm[bi]] = ar
        # every unmasked row must sort into the emitted prefix
        if not (ranks[bi][msk[bi]] < nbu * BIN).all():
            raise RuntimeError(
                f"batch {bi}: unmasked rows beyond {nbu} sorted bins; "
                f"increase NBU")

    # ---- modM: main pipeline with exact ranks ----
    full = {"m": mf, "rank": ranks.reshape(Btot * NP, 1)}
    for h in range(nb):
        full[f"x{h}"] = xb_dev[h]
    outM = shardedM(*[full[n] for n in in_namesM], *dev_zerosM)
    resM = dict(zip(out_namesM, outM))

    # ---- fetch + dequantize + scatter back to input order; the dequant of
    # one output overlaps the fetch of the next ----
    out = np.zeros((Btot, NP, F), np.float32)
    with cf.ThreadPoolExecutor(2) as ex:
        fosc = {b: ex.submit(np.asarray, resM[f"osc{b}"]) for b in range(nb)}
        fout = {b: ex.submit(np.asarray, resM[f"out{b}"]) for b in range(nb)}
        for b in range(nb):
            q_all = fout[b].result().reshape(n_cores, nbu * BIN, F)
            sc_all = fosc[b].result().reshape(n_cores, nbu * BIN, 1)
            for core in range(n_cores):
                gb = core * nb + b
                ids = perm[gb, :nbu * BIN]
                out[gb, ids] = q_all[core].astype(np.float32) * sc_all[core]
    return out, None


def kernel(**inputs):
    out, _ = run(inputs, nb=2, nch=100, n_cores=8)
    return out
